# revision 15
# baseline (speedup 1.0000x reference)
"""Multi-head attention (B=4, S=2048, D=1024, H=16, Dh=64) on 8 TRN2 NeuronCores.

Sharding: core c -> batch b = c//2, head-group g = c%2 (8 heads, output cols
g*512:(g+1)*512).  Host ships x pre-transposed ([D, S]) and weights in bf16;
each core runs attention for its (batch, 8 heads) slice; host concatenates the
per-core [2048, 512] outputs.

v3 structure (vs v2):
- DMA: ~12 fat dma_starts on 3 engine rings (sync/scalar/gpsimd) instead of
  81 small ones on 2 rings -- descriptor-gen (618ns per start, serialized
  per ring) stops gating the load; x lands in 4 column pieces so the pair-0
  projections start ~6us in.
- PE warms up on a zero dummy tile (no x dependency) so HAM hits 2.4GHz
  before the first projection.
- Scores per chunk: 2 concurrent K=64 matmuls (PE row groups 0-63/64-127),
  N=512 each, into separate PSUM banks of one [128,1024] tile.
- AV is software-pipelined one chunk late: AV(c) is emitted after exp(c+1),
  so it never heads-of-line-blocks the PE queue waiting on exp(c); the
  previous segment's last AV + accumulator copy-out ride the same mechanism.
- Projection filler is paced by explicit per-generator deadline windows
  (chunk index within the segment), balanced so no segment except (0,0)
  exceeds the exp-rate PE budget; v projections run inline in segment (0,0).
- Tails (transpose/normalize of finished segments) drain via a dynamic
  spreader, never at segment boundaries.
"""

import numpy as np
import ml_dtypes
from collections import deque
from contextlib import ExitStack

import concourse.bass as bass
import concourse.bacc as bacc
import concourse.mybir as mybir
import concourse.tile as tile
from concourse.bass_utils import run_bass_kernel_spmd
from concourse.masks import make_identity

F32 = mybir.dt.float32
BF16 = mybir.dt.bfloat16

B, S, D = 4, 2048, 1024
H, DH = 16, 64
N_CORES = 8
HPC = 8          # heads per core
DPC = HPC * DH   # output cols per core = 512
SCALE = 1.0 / 32.0  # 1/sqrt(D)

KD = D // 128    # 8 contraction chunks over d_in
NS = S // 128    # 16 sk chunks
MB = HPC // 2    # 4 head pairs
NT4 = S // 512   # 4 sq tiles of 512

_CACHE = {}


def _build_program():
    nc = bacc.Bacc("TRN2", target_bir_lowering=False, debug=False)

    xt_ext = nc.dram_tensor("xt", [D, S], BF16, kind="ExternalInput").ap()
    wq_ext = nc.dram_tensor("wq", [D, DPC], BF16, kind="ExternalInput").ap()
    wk_ext = nc.dram_tensor("wk", [D, DPC], BF16, kind="ExternalInput").ap()
    wv_ext = nc.dram_tensor("wv", [D, DPC], BF16, kind="ExternalInput").ap()
    bq_ext = nc.dram_tensor("bq", [DPC], F32, kind="ExternalInput").ap()
    bk_ext = nc.dram_tensor("bk", [DPC], F32, kind="ExternalInput").ap()
    bv_ext = nc.dram_tensor("bv", [DPC], F32, kind="ExternalInput").ap()
    out_ext = nc.dram_tensor("out", [S, DPC], F32, kind="ExternalOutput").ap()

    with tile.TileContext(nc, pool_alloc_mode="queue") as tc, ExitStack() as ctx:
        singles = ctx.enter_context(tc.tile_pool(name="singles", bufs=1))

        # --- biases on the gpsimd (software-DGE) ring so the sync HW ring
        # starts generating x descriptors immediately ---
        bqm = singles.tile([128, MB], F32, tag="bqm")
        nc.gpsimd.dma_start(out=bqm, in_=bq_ext.rearrange("(m p) -> p m", p=128))
        bkm = singles.tile([128, MB], F32, tag="bkm")
        nc.gpsimd.dma_start(out=bkm, in_=bk_ext.rearrange("(m p) -> p m", p=128))
        bv_f32 = singles.tile([1, DPC], F32, tag="bv_f32")
        nc.gpsimd.dma_start(out=bv_f32, in_=bv_ext.rearrange("(o n) -> o n", o=1))
        bv_row = singles.tile([1, DPC], BF16, tag="bv_row")
        nc.vector.tensor_copy(bv_row, bv_f32)

        # --- fat input DMAs on 3 rings; arrival waves:
        #   wave 1 (needed ~6us):  x piece0 | wq first half | wk first half
        #   wave 2 (needed ~20us): x piece1 | wq h2 + wv    | wk h2 + x piece2
        #   wave 3 (needed ~35us): x piece3
        xT = singles.tile([128, KD, S], BF16, tag="xT")
        xt_r = xt_ext.rearrange("(j p) s -> p j s", p=128)
        wq_sb = singles.tile([128, KD, DPC], BF16, tag="wq_sb")
        wk_sb = singles.tile([128, KD, DPC], BF16, tag="wk_sb")
        wv_sb = singles.tile([128, KD, DPC], BF16, tag="wv_sb")
        wq_r = wq_ext.rearrange("(k p) n -> p k n", p=128)
        wk_r = wk_ext.rearrange("(k p) n -> p k n", p=128)
        wv_r = wv_ext.rearrange("(k p) n -> p k n", p=128)

        # identity/memsets first so no engine's iota queues behind a fat
        # DMA-descriptor-generation instruction
        identity = singles.tile([128, 128], BF16, tag="identity")
        make_identity(nc, identity)
        ones_row = singles.tile([1, 128], BF16, tag="ones_row")
        nc.vector.memset(ones_row, 1.0)
        dummy = singles.tile([128, 512], BF16, tag="dummy")
        nc.vector.memset(dummy, 0.0)

        # The load phase is DMA-descriptor-rate bound: keep descriptor runs
        # at >=1KB and the count low.  x goes in two 1024-col pieces (2KB
        # contiguous DRAM runs); weights as whole-matrix DMAs (1KB runs).
        # ring: sync -- first 512 cols alone so q00/k00 unblock earliest
        nc.sync.dma_start(out=xT[:, :, 0:512], in_=xt_r[:, :, 0:512])
        nc.sync.dma_start(out=xT[:, :, 512:1024], in_=xt_r[:, :, 512:1024])
        nc.sync.dma_start(out=xT[:, :, 1024:2048], in_=xt_r[:, :, 1024:2048])
        # ring: scalar (idle until the first exp anyway)
        nc.scalar.dma_start(out=wq_sb, in_=wq_r)
        nc.scalar.dma_start(out=wv_sb, in_=wv_r)
        # ring: gpsimd
        nc.gpsimd.dma_start(out=wk_sb, in_=wk_r)

        # --- persistent sbuf tensors: pair-packed qT/kT (rows 0-63 even
        # head's projection cols, 64-127 odd head's), v natural + ones col ---
        qT = [singles.tile([128, S], BF16, tag=f"qT{m}", name=f"qT{m}") for m in range(MB)]
        kT = [singles.tile([128, S], BF16, tag=f"kT{m}", name=f"kT{m}") for m in range(MB)]
        vsb = [singles.tile([128, HPC, DH + 1], BF16, tag=f"v{i}", name=f"v{i}") for i in range(NS)]
        out_full = [singles.tile([128, DPC], F32, tag=f"of{i}", name=f"of{i}") for i in range(NS)]

        # --- psum pools: scores 2x[128,1024]f32 (4 banks) + shared
        # accumulator/projection pool (4 banks) ---
        o_psum = ctx.enter_context(tc.tile_pool(name="o_psum", bufs=2, space="PSUM"))
        s_psum = ctx.enter_context(tc.tile_pool(name="s_psum", bufs=2, space="PSUM"))

        e_pool = ctx.enter_context(tc.tile_pool(name="e_pool", bufs=6))
        cp_sb = ctx.enter_context(tc.tile_pool(name="cp_sb", bufs=4))
        ot_sb = ctx.enter_context(tc.tile_pool(name="ot_sb", bufs=8))

        # warm the PE clock (HAM) on the zero dummy (no DMA dependency): the
        # SHORT window needs ~3.4us of sustained activity, 8 cold N=512 MMs
        warm = o_psum.tile([128, 512], F32, tag="po", name="warm")
        for _ in range(8):
            nc.tensor.matmul(warm, lhsT=identity, rhs=dummy, start=True, stop=True)

        # bv replicated across partitions (one K=1 matmul): the v bias is
        # applied once in the tail -- attention weights sum to 1 exactly, so
        # out = sum(attn (v'+bv)) = sum(attn v') + bv
        bvp = o_psum.tile([128, DPC], F32, tag="po", name="bvp")
        nc.tensor.matmul(bvp, lhsT=ones_row, rhs=bv_row, start=True, stop=True)
        bv_bc = singles.tile([128, DPC], BF16, tag="bv_bc")
        nc.vector.tensor_copy(bv_bc, bvp)

        def gen_q_proj(m, n):
            """q projection for pair m, seq cols n*512:(n+1)*512."""
            sl = slice(n * 512, (n + 1) * 512)
            ps = o_psum.tile([128, 512], F32, tag="po", name=f"ppq{m}_{n}")
            for k in range(KD):
                nc.tensor.matmul(
                    ps,
                    lhsT=wq_sb[:, k, m * 128:(m + 1) * 128],
                    rhs=xT[:, k, sl],
                    start=(k == 0),
                    stop=(k == KD - 1),
                )
                if k % 2 == 1:
                    yield
            nc.vector.tensor_scalar_add(qT[m][:, sl], ps, bqm[:, m:m + 1])

        def gen_k_proj(m, n):
            sl = slice(n * 512, (n + 1) * 512)
            ps = o_psum.tile([128, 512], F32, tag="po", name=f"ppk{m}_{n}")
            for k in range(KD):
                nc.tensor.matmul(
                    ps,
                    lhsT=wk_sb[:, k, m * 128:(m + 1) * 128],
                    rhs=xT[:, k, sl],
                    start=(k == 0),
                    stop=(k == KD - 1),
                )
                if k % 2 == 1:
                    yield
            nc.vector.tensor_scalar_add(kT[m][:, sl], ps, bkm[:, m:m + 1])

        def gen_v_proj(i):
            ps = o_psum.tile([128, 512], F32, tag="po", name=f"vp{i}")
            for k in range(KD):
                nc.tensor.matmul(
                    ps,
                    lhsT=xT[:, k, i * 128:(i + 1) * 128],
                    rhs=wv_sb[:, k, :],
                    start=(k == 0),
                    stop=(k == KD - 1),
                )
                if k % 2 == 1:
                    yield
            nc.vector.tensor_copy(
                vsb[i][:, :, 0:DH], ps.rearrange("p (h d) -> p h d", h=HPC)
            )
            nc.vector.memset(vsb[i][:, :, DH:DH + 1], 1.0)

        out_dma_n = [0]

        def emit_out_dma(i, tail=False):
            # sync ring during the attention stream; alternate sync/gpsimd
            # for the final drain (never scalar: it would steal exp time)
            eng = nc.gpsimd if (tail and out_dma_n[0] % 2) else nc.sync
            out_dma_n[0] += 1
            eng.dma_start(out=out_ext[i * 128:(i + 1) * 128, :], in_=out_full[i])

        def emit_exp(e, psc):
            # raw InstActivation with immediate bias/scale: skips the
            # per-partition bias-AP read the bass helper forces for Exp
            imm = lambda v: mybir.ImmediateValue(dtype=mybir.dt.float32, value=v)
            return nc.scalar.add_instruction(
                mybir.InstActivation(
                    name=nc.get_next_instruction_name(),
                    func=mybir.ActivationFunctionType.Exp,
                    ins=[nc.scalar.lower_ap(psc), imm(0.0), imm(SCALE), imm(0.0)],
                    outs=[nc.scalar.lower_ap(e)],
                )
            )

        def gen_tail(m, t4, cp):
            """Transpose/normalize pieces for segment (m, t4), reading the
            sbuf copy cp ([65,1024]: h0 | h1); one quantum per yield."""
            for c2 in range(4):
                for j in range(2):
                    pt = o_psum.tile([128, 65], BF16, tag="po", name=f"pt{m}_{t4}_{c2}_{j}")
                    nc.tensor.transpose(
                        pt,
                        cp[:, j * 512 + c2 * 128:j * 512 + (c2 + 1) * 128],
                        identity[0:65, 0:65],
                    )
                    ot = ot_sb.tile([128, 65], BF16, tag="ot", name=f"ot{m}_{t4}_{c2}_{j}")
                    nc.vector.tensor_copy(ot, pt)
                    rc = ot_sb.tile([128, 1], F32, tag="rc", name=f"rc{m}_{t4}_{c2}_{j}")
                    nc.vector.reciprocal(rc, ot[:, DH:DH + 1])
                    nc.vector.scalar_tensor_tensor(
                        out_full[t4 * 4 + c2][:, (2 * m + j) * DH:(2 * m + j + 1) * DH],
                        ot[:, 0:DH],
                        rc,
                        bv_bc[:, (2 * m + j) * DH:(2 * m + j + 1) * DH],
                        mybir.AluOpType.mult,
                        mybir.AluOpType.add,
                    )
                    if m == MB - 1 and j == 1:
                        emit_out_dma(t4 * 4 + c2, tail=final[0])
                    yield
            return

        # --- prelude: only q00/k00 gate the first chunk's scores ---
        for _ in gen_q_proj(0, 0):
            pass
        for _ in gen_k_proj(0, 0):
            pass

        # --- filler pacing: per-segment schedule of (due_chunk, fn), built
        # from explicit windows; tails drain via a dynamic spreader ---
        tails = []
        final = [False]
        tail_gen = [None, 0]  # generator, yields-left

        def drain_tail_step(k):
            for _ in range(k):
                if tail_gen[0] is None:
                    if not tails:
                        return
                    tail_gen[0] = tails.pop(0)
                    tail_gen[1] = 8
                try:
                    if tail_gen[1] > 0:
                        tail_gen[0].__next__()
                        tail_gen[1] -= 1
                    else:
                        deque(tail_gen[0], maxlen=0)
                        tail_gen[0] = None
                except StopIteration:
                    tail_gen[0] = None

        def make_sched(m, t4):
            sched = deque()

            def push(g, c0, c1, npops=4):
                total = npops + 1
                for i in range(total):
                    due = c0 + ((c1 - c0) * i) // (total - 1)
                    if i < npops:
                        sched.append((due, g.__next__))
                    else:
                        sched.append((due, lambda gg=g: deque(gg, maxlen=0)))

            # NOTE: every segment of pair m reads ALL of kT[m] (chunk c reads
            # sk cols c*128:(c+1)*128), so k(m, n) must complete inside the
            # FIRST segment of pair m by chunk 4n.  q(m, t4) is needed from
            # segment (m, t4) on.
            if t4 == 0:
                if m == 0:
                    push(gen_k_proj(0, 1), 0, 2)
                    push(gen_k_proj(0, 2), 3, 6)
                    push(gen_k_proj(0, 3), 7, 10)
                    push(gen_q_proj(0, 1), 11, 14)
                else:
                    push(gen_k_proj(m, 3), 1, 9)
                    push(gen_q_proj(m, 1), 10, 14)
            elif t4 == 1:
                if m == 0:
                    push(gen_q_proj(0, 2), 1, 8)
                else:
                    push(gen_q_proj(m, 2), 1, 7)
                    if m + 1 < MB:
                        push(gen_k_proj(m + 1, 1), 8, 15)
            elif t4 == 2:
                if m == 0:
                    push(gen_q_proj(0, 3), 1, 5)
                    push(gen_k_proj(1, 1), 6, 10)
                    push(gen_k_proj(1, 2), 11, 15)
                else:
                    push(gen_q_proj(m, 3), 1, 7)
                    if m + 1 < MB:
                        push(gen_k_proj(m + 1, 2), 8, 15)
            else:
                if m + 1 < MB:
                    push(gen_q_proj(m + 1, 0), 1, 6)
                    push(gen_k_proj(m + 1, 0), 7, 13)
            return sched

        pending = [None]

        for m in range(MB):
            for t4 in range(NT4):
                sched = make_sched(m, t4)
                po = o_psum.tile([128, 1024], F32, tag="po", name=f"po{m}_{t4}")
                sq = slice(t4 * 512, (t4 + 1) * 512)
                first_seg = (m == 0 and t4 == 0)

                def emit_av(c, e, po=po, m=m):
                    nc.tensor.matmul(
                        po[0:65, 0:512],
                        lhsT=vsb[c][:, 2 * m, :],
                        rhs=e[:, 0:512],
                        start=(c == 0), stop=(c == NS - 1),
                    )
                    nc.tensor.matmul(
                        po[0:65, 512:1024],
                        lhsT=vsb[c][:, 2 * m + 1, :],
                        rhs=e[:, 512:1024],
                        start=(c == 0), stop=(c == NS - 1),
                    )

                def finish_segment(po=po, m=m, t4=t4):
                    cp = cp_sb.tile([65, 1024], BF16, tag="cp", name=f"cp{m}_{t4}")
                    nc.vector.tensor_copy(cp, po[0:65, :])
                    tails.append(gen_tail(m, t4, cp))

                # two chunks per pipeline step: scores x2 (one K=64 island),
                # exp x2, then the previous step's AVs x4 (one full-array
                # island) -- halves the K64<->K128 PE mode-switch penalty
                for c0 in range(0, NS, 2):
                    es = []
                    for c in (c0, c0 + 1):
                        psc = s_psum.tile([128, 1024], F32, tag="psc", name=f"ps{m}_{t4}_{c}")
                        nc.tensor.matmul(
                            psc[:, 0:512],
                            lhsT=kT[m][0:64, c * 128:(c + 1) * 128],
                            rhs=qT[m][0:64, sq],
                            start=True, stop=True,
                        )
                        nc.tensor.matmul(
                            psc[:, 512:1024],
                            lhsT=kT[m][64:128, c * 128:(c + 1) * 128],
                            rhs=qT[m][64:128, sq],
                            start=True, stop=True,
                        )
                        e = e_pool.tile([128, 1024], BF16, tag="e", name=f"e{m}_{t4}_{c}")
                        emit_exp(e, psc)
                        es.append(e)
                    # deferred AVs from the previous step (or the previous
                    # segment's last step + accumulator copy-out): their exps
                    # are done, so they never stall the PE queue
                    if pending[0] is not None:
                        pending[0]()
                        pending[0] = None
                    if first_seg:
                        # v(c0), v(c0+1) land ahead of their deferred AVs
                        for i in (c0, c0 + 1):
                            for _ in gen_v_proj(i):
                                pass
                    while sched and sched[0][0] <= c0 + 1:
                        sched.popleft()[1]()
                    if not first_seg and 1 <= c0 <= 13:
                        drain_tail_step(2 if len(tails) < 2 else 4)
                    if c0 < NS - 2:
                        def _p(es=es, c0=c0, emit_av=emit_av):
                            emit_av(c0, es[0])
                            emit_av(c0 + 1, es[1])
                        pending[0] = _p
                    else:
                        # retire the segment's last AVs inside the segment:
                        # the preceding v/filler PE work covers exp(14),(15)'s
                        # latency, and the next segment's first scores then
                        # enter a clean PE queue
                        emit_av(NS - 2, es[0])
                        emit_av(NS - 1, es[1])
                        finish_segment()

        if pending[0] is not None:
            pending[0]()
            pending[0] = None
        final[0] = True
        while tail_gen[0] is not None or tails:
            drain_tail_step(4)

    nc.compile()
    return nc


def _get_program():
    if "nc" not in _CACHE:
        _CACHE["nc"] = _build_program()
    return _CACHE["nc"]


def kernel(x, Wq, bq, Wk, bk, Wv, bv, _trace=False):
    bf = ml_dtypes.bfloat16
    x = np.asarray(x, dtype=np.float32)
    Wq = np.asarray(Wq, dtype=np.float32)
    Wk = np.asarray(Wk, dtype=np.float32)
    Wv = np.asarray(Wv, dtype=np.float32)
    bq = np.ascontiguousarray(np.asarray(bq, dtype=np.float32))
    bk = np.ascontiguousarray(np.asarray(bk, dtype=np.float32))
    bv = np.ascontiguousarray(np.asarray(bv, dtype=np.float32))

    nc = _get_program()

    in_maps = []
    for c in range(N_CORES):
        b, g = c // 2, c % 2
        cols = slice(g * DPC, (g + 1) * DPC)
        in_maps.append(
            {
                "xt": np.ascontiguousarray(x[b].T.astype(bf)),
                "wq": np.ascontiguousarray(Wq[:, cols].astype(bf)),
                "wk": np.ascontiguousarray(Wk[:, cols].astype(bf)),
                "wv": np.ascontiguousarray(Wv[:, cols].astype(bf)),
                "bq": np.ascontiguousarray(bq[cols]),
                "bk": np.ascontiguousarray(bk[cols]),
                "bv": np.ascontiguousarray(bv[cols]),
            }
        )

    res = run_bass_kernel_spmd(nc, in_maps, core_ids=list(range(N_CORES)), trace=_trace)
    _CACHE["last_results"] = res

    out = np.empty((B, S, D), dtype=np.float32)
    for c in range(N_CORES):
        b, g = c // 2, c % 2
        out[b, :, g * DPC:(g + 1) * DPC] = res.results[c]["out"]
    return out


# revision 16
# speedup vs baseline: 1.0845x; 1.0845x over previous
"""Multi-head attention (B=4, S=2048, D=1024, H=16, Dh=64) on 8 TRN2 NeuronCores.

Sharding: core c -> batch b = c//2, head-group g = c%2 (8 heads, output cols
g*512:(g+1)*512).  Host ships x pre-transposed ([D, S]) and weights in bf16;
each core runs attention for its (batch, 8 heads) slice; host concatenates the
per-core [2048, 512] outputs.

v3 structure (vs v2):
- DMA: ~12 fat dma_starts on 3 engine rings (sync/scalar/gpsimd) instead of
  81 small ones on 2 rings -- descriptor-gen (618ns per start, serialized
  per ring) stops gating the load; x lands in 4 column pieces so the pair-0
  projections start ~6us in.
- PE warms up on a zero dummy tile (no x dependency) so HAM hits 2.4GHz
  before the first projection.
- Scores per chunk: 2 concurrent K=64 matmuls (PE row groups 0-63/64-127),
  N=512 each, into separate PSUM banks of one [128,1024] tile.
- AV is software-pipelined one chunk late: AV(c) is emitted after exp(c+1),
  so it never heads-of-line-blocks the PE queue waiting on exp(c); the
  previous segment's last AV + accumulator copy-out ride the same mechanism.
- Projection filler is paced by explicit per-generator deadline windows
  (chunk index within the segment), balanced so no segment except (0,0)
  exceeds the exp-rate PE budget; v projections run inline in segment (0,0).
- Tails (transpose/normalize of finished segments) drain via a dynamic
  spreader, never at segment boundaries.
"""

import numpy as np
import ml_dtypes
from collections import deque
from contextlib import ExitStack

import concourse.bass as bass
import concourse.bacc as bacc
import concourse.mybir as mybir
import concourse.tile as tile
from concourse.bass_utils import run_bass_kernel_spmd
from concourse.masks import make_identity

F32 = mybir.dt.float32
BF16 = mybir.dt.bfloat16

B, S, D = 4, 2048, 1024
H, DH = 16, 64
N_CORES = 8
HPC = 8          # heads per core
DPC = HPC * DH   # output cols per core = 512
SCALE = 1.0 / 32.0  # 1/sqrt(D)

KD = D // 128    # 8 contraction chunks over d_in
NS = S // 128    # 16 sk chunks
MB = HPC // 2    # 4 head pairs
NT4 = S // 512   # 4 sq tiles of 512

_CACHE = {}


def _build_program():
    nc = bacc.Bacc("TRN2", target_bir_lowering=False, debug=False)

    xt_ext = nc.dram_tensor("xt", [D, S], BF16, kind="ExternalInput").ap()
    wq_ext = nc.dram_tensor("wq", [D, DPC], BF16, kind="ExternalInput").ap()
    wk_ext = nc.dram_tensor("wk", [D, DPC], BF16, kind="ExternalInput").ap()
    wv_ext = nc.dram_tensor("wv", [D, DPC], BF16, kind="ExternalInput").ap()
    bq_ext = nc.dram_tensor("bq", [DPC], F32, kind="ExternalInput").ap()
    bk_ext = nc.dram_tensor("bk", [DPC], F32, kind="ExternalInput").ap()
    bv_ext = nc.dram_tensor("bv", [DPC], F32, kind="ExternalInput").ap()
    out_ext = nc.dram_tensor("out", [S, DPC], F32, kind="ExternalOutput").ap()

    with tile.TileContext(nc, pool_alloc_mode="queue") as tc, ExitStack() as ctx:
        singles = ctx.enter_context(tc.tile_pool(name="singles", bufs=1))

        # --- biases: one consolidated DMA each (tiny, land instantly) ---
        bqm = singles.tile([128, MB], F32, tag="bqm")
        nc.sync.dma_start(out=bqm, in_=bq_ext.rearrange("(m p) -> p m", p=128))
        bkm = singles.tile([128, MB], F32, tag="bkm")
        nc.sync.dma_start(out=bkm, in_=bk_ext.rearrange("(m p) -> p m", p=128))
        bv_f32 = singles.tile([1, DPC], F32, tag="bv_f32")
        nc.sync.dma_start(out=bv_f32, in_=bv_ext.rearrange("(o n) -> o n", o=1))
        bv_row = singles.tile([1, DPC], BF16, tag="bv_row")
        nc.vector.tensor_copy(bv_row, bv_f32)

        # --- fat input DMAs on 3 rings; arrival waves:
        #   wave 1 (needed ~6us):  x piece0 | wq first half | wk first half
        #   wave 2 (needed ~20us): x piece1 | wq h2 + wv    | wk h2 + x piece2
        #   wave 3 (needed ~35us): x piece3
        xT = singles.tile([128, KD, S], BF16, tag="xT")
        xt_r = xt_ext.rearrange("(j p) s -> p j s", p=128)
        wq_sb = singles.tile([128, KD, DPC], BF16, tag="wq_sb")
        wk_sb = singles.tile([128, KD, DPC], BF16, tag="wk_sb")
        wv_sb = singles.tile([128, KD, DPC], BF16, tag="wv_sb")
        wq_r = wq_ext.rearrange("(k p) n -> p k n", p=128)
        wk_r = wk_ext.rearrange("(k p) n -> p k n", p=128)
        wv_r = wv_ext.rearrange("(k p) n -> p k n", p=128)

        # identity/memsets first so no engine's iota queues behind a fat
        # DMA-descriptor-generation instruction
        identity = singles.tile([128, 128], BF16, tag="identity")
        make_identity(nc, identity)
        ones_row = singles.tile([1, 128], BF16, tag="ones_row")
        nc.vector.memset(ones_row, 1.0)
        dummy = singles.tile([128, 512], BF16, tag="dummy")
        nc.vector.memset(dummy, 0.0)

        # The load phase is DMA-descriptor-rate bound: keep descriptor runs
        # at >=1KB and the count low.  x goes in two 1024-col pieces (2KB
        # contiguous DRAM runs); weights as whole-matrix DMAs (1KB runs).
        # ring: sync -- first 512 cols alone so q00/k00 unblock earliest
        nc.sync.dma_start(out=xT[:, :, 0:512], in_=xt_r[:, :, 0:512])
        nc.sync.dma_start(out=xT[:, :, 512:1024], in_=xt_r[:, :, 512:1024])
        nc.sync.dma_start(out=xT[:, :, 1024:2048], in_=xt_r[:, :, 1024:2048])
        # ring: scalar (idle until the first exp anyway)
        nc.scalar.dma_start(out=wq_sb, in_=wq_r)
        nc.scalar.dma_start(out=wv_sb, in_=wv_r)
        # ring: gpsimd
        nc.gpsimd.dma_start(out=wk_sb, in_=wk_r)

        # --- persistent sbuf tensors: pair-packed qT/kT (rows 0-63 even
        # head's projection cols, 64-127 odd head's), v natural + ones col ---
        qT = [singles.tile([128, S], BF16, tag=f"qT{m}", name=f"qT{m}") for m in range(MB)]
        kT = [singles.tile([128, S], BF16, tag=f"kT{m}", name=f"kT{m}") for m in range(MB)]
        vsb = [singles.tile([128, HPC, DH + 1], BF16, tag=f"v{i}", name=f"v{i}") for i in range(NS)]
        out_full = [singles.tile([128, DPC], F32, tag=f"of{i}", name=f"of{i}") for i in range(NS)]

        # --- psum pools: scores 2x[128,1024]f32 (4 banks) + shared
        # accumulator/projection pool (4 banks) ---
        o_psum = ctx.enter_context(tc.tile_pool(name="o_psum", bufs=2, space="PSUM"))
        s_psum = ctx.enter_context(tc.tile_pool(name="s_psum", bufs=2, space="PSUM"))

        e_pool = ctx.enter_context(tc.tile_pool(name="e_pool", bufs=6))
        cp_sb = ctx.enter_context(tc.tile_pool(name="cp_sb", bufs=4))
        ot_sb = ctx.enter_context(tc.tile_pool(name="ot_sb", bufs=8))

        # warm the PE clock (HAM) on the zero dummy (no DMA dependency): the
        # SHORT window needs ~3.4us of sustained activity, 8 cold N=512 MMs
        warm = o_psum.tile([128, 512], F32, tag="po", name="warm")
        for _ in range(8):
            nc.tensor.matmul(warm, lhsT=identity, rhs=dummy, start=True, stop=True)

        # bv replicated across partitions (one K=1 matmul): the v bias is
        # applied once in the tail -- attention weights sum to 1 exactly, so
        # out = sum(attn (v'+bv)) = sum(attn v') + bv
        bvp = o_psum.tile([128, DPC], F32, tag="po", name="bvp")
        nc.tensor.matmul(bvp, lhsT=ones_row, rhs=bv_row, start=True, stop=True)
        bv_bc = singles.tile([128, DPC], BF16, tag="bv_bc")
        nc.vector.tensor_copy(bv_bc, bvp)

        def gen_q_proj(m, n):
            """q projection for pair m, seq cols n*512:(n+1)*512."""
            sl = slice(n * 512, (n + 1) * 512)
            ps = o_psum.tile([128, 512], F32, tag="po", name=f"ppq{m}_{n}")
            for k in range(KD):
                nc.tensor.matmul(
                    ps,
                    lhsT=wq_sb[:, k, m * 128:(m + 1) * 128],
                    rhs=xT[:, k, sl],
                    start=(k == 0),
                    stop=(k == KD - 1),
                )
                if k % 2 == 1:
                    yield
            nc.vector.tensor_scalar_add(qT[m][:, sl], ps, bqm[:, m:m + 1])

        def gen_k_proj(m, n):
            sl = slice(n * 512, (n + 1) * 512)
            ps = o_psum.tile([128, 512], F32, tag="po", name=f"ppk{m}_{n}")
            for k in range(KD):
                nc.tensor.matmul(
                    ps,
                    lhsT=wk_sb[:, k, m * 128:(m + 1) * 128],
                    rhs=xT[:, k, sl],
                    start=(k == 0),
                    stop=(k == KD - 1),
                )
                if k % 2 == 1:
                    yield
            nc.vector.tensor_scalar_add(kT[m][:, sl], ps, bkm[:, m:m + 1])

        def gen_v_proj(i):
            ps = o_psum.tile([128, 512], F32, tag="po", name=f"vp{i}")
            for k in range(KD):
                nc.tensor.matmul(
                    ps,
                    lhsT=xT[:, k, i * 128:(i + 1) * 128],
                    rhs=wv_sb[:, k, :],
                    start=(k == 0),
                    stop=(k == KD - 1),
                )
                if k % 2 == 1:
                    yield
            nc.vector.tensor_copy(
                vsb[i][:, :, 0:DH], ps.rearrange("p (h d) -> p h d", h=HPC)
            )
            nc.vector.memset(vsb[i][:, :, DH:DH + 1], 1.0)

        out_dma_n = [0]

        def emit_out_dma(i, tail=False):
            # sync ring during the attention stream; alternate sync/gpsimd
            # for the final drain (never scalar: it would steal exp time)
            eng = nc.gpsimd if (tail and out_dma_n[0] % 2) else nc.sync
            out_dma_n[0] += 1
            eng.dma_start(out=out_ext[i * 128:(i + 1) * 128, :], in_=out_full[i])

        def emit_exp(e, psc):
            # raw InstActivation with immediate bias/scale: skips the
            # per-partition bias-AP read the bass helper forces for Exp
            imm = lambda v: mybir.ImmediateValue(dtype=mybir.dt.float32, value=v)
            return nc.scalar.add_instruction(
                mybir.InstActivation(
                    name=nc.get_next_instruction_name(),
                    func=mybir.ActivationFunctionType.Exp,
                    ins=[nc.scalar.lower_ap(psc), imm(0.0), imm(SCALE), imm(0.0)],
                    outs=[nc.scalar.lower_ap(e)],
                )
            )

        def gen_tail(m, t4, cp):
            """Transpose/normalize pieces for segment (m, t4), reading the
            sbuf copy cp ([65,1024]: h0 | h1); one quantum per yield."""
            for c2 in range(4):
                for j in range(2):
                    pt = o_psum.tile([128, 65], BF16, tag="po", name=f"pt{m}_{t4}_{c2}_{j}")
                    nc.tensor.transpose(
                        pt,
                        cp[:, j * 512 + c2 * 128:j * 512 + (c2 + 1) * 128],
                        identity[0:65, 0:65],
                    )
                    ot = ot_sb.tile([128, 65], BF16, tag="ot", name=f"ot{m}_{t4}_{c2}_{j}")
                    nc.vector.tensor_copy(ot, pt)
                    rc = ot_sb.tile([128, 1], F32, tag="rc", name=f"rc{m}_{t4}_{c2}_{j}")
                    nc.vector.reciprocal(rc, ot[:, DH:DH + 1])
                    nc.vector.scalar_tensor_tensor(
                        out_full[t4 * 4 + c2][:, (2 * m + j) * DH:(2 * m + j + 1) * DH],
                        ot[:, 0:DH],
                        rc,
                        bv_bc[:, (2 * m + j) * DH:(2 * m + j + 1) * DH],
                        mybir.AluOpType.mult,
                        mybir.AluOpType.add,
                    )
                    if m == MB - 1 and j == 1:
                        emit_out_dma(t4 * 4 + c2, tail=final[0])
                    yield
            return

        # --- prelude: only q00/k00 gate the first chunk's scores ---
        for _ in gen_q_proj(0, 0):
            pass
        for _ in gen_k_proj(0, 0):
            pass

        # --- filler pacing: per-segment schedule of (due_chunk, fn), built
        # from explicit windows; tails drain via a dynamic spreader ---
        tails = []
        final = [False]
        tail_gen = [None, 0]  # generator, yields-left

        def drain_tail_step(k):
            for _ in range(k):
                if tail_gen[0] is None:
                    if not tails:
                        return
                    tail_gen[0] = tails.pop(0)
                    tail_gen[1] = 8
                try:
                    if tail_gen[1] > 0:
                        tail_gen[0].__next__()
                        tail_gen[1] -= 1
                    else:
                        deque(tail_gen[0], maxlen=0)
                        tail_gen[0] = None
                except StopIteration:
                    tail_gen[0] = None

        def make_sched(m, t4):
            sched = deque()

            def push(g, c0, c1, npops=4):
                total = npops + 1
                for i in range(total):
                    due = c0 + ((c1 - c0) * i) // (total - 1)
                    if i < npops:
                        sched.append((due, g.__next__))
                    else:
                        sched.append((due, lambda gg=g: deque(gg, maxlen=0)))

            # NOTE: every segment of pair m reads ALL of kT[m] (chunk c reads
            # sk cols c*128:(c+1)*128), so k(m, n) must complete inside the
            # FIRST segment of pair m by chunk 4n.  q(m, t4) is needed from
            # segment (m, t4) on.
            if t4 == 0:
                if m == 0:
                    push(gen_k_proj(0, 1), 0, 2)
                    push(gen_k_proj(0, 2), 3, 6)
                    push(gen_k_proj(0, 3), 7, 10)
                    push(gen_q_proj(0, 1), 11, 14)
                else:
                    push(gen_k_proj(m, 3), 1, 9)
                    push(gen_q_proj(m, 1), 10, 14)
            elif t4 == 1:
                if m == 0:
                    push(gen_q_proj(0, 2), 1, 8)
                else:
                    push(gen_q_proj(m, 2), 1, 7)
                    if m + 1 < MB:
                        push(gen_k_proj(m + 1, 1), 8, 15)
            elif t4 == 2:
                if m == 0:
                    push(gen_q_proj(0, 3), 1, 5)
                    push(gen_k_proj(1, 1), 6, 10)
                    push(gen_k_proj(1, 2), 11, 15)
                else:
                    push(gen_q_proj(m, 3), 1, 7)
                    if m + 1 < MB:
                        push(gen_k_proj(m + 1, 2), 8, 15)
            else:
                if m + 1 < MB:
                    push(gen_q_proj(m + 1, 0), 1, 6)
                    push(gen_k_proj(m + 1, 0), 7, 13)
            return sched

        pending = [None]

        for m in range(MB):
            for t4 in range(NT4):
                sched = make_sched(m, t4)
                po = o_psum.tile([128, 1024], F32, tag="po", name=f"po{m}_{t4}")
                sq = slice(t4 * 512, (t4 + 1) * 512)
                first_seg = (m == 0 and t4 == 0)

                def emit_av(c, e, po=po, m=m):
                    nc.tensor.matmul(
                        po[0:65, 0:512],
                        lhsT=vsb[c][:, 2 * m, :],
                        rhs=e[:, 0:512],
                        start=(c == 0), stop=(c == NS - 1),
                    )
                    nc.tensor.matmul(
                        po[0:65, 512:1024],
                        lhsT=vsb[c][:, 2 * m + 1, :],
                        rhs=e[:, 512:1024],
                        start=(c == 0), stop=(c == NS - 1),
                    )

                def finish_segment(po=po, m=m, t4=t4):
                    cp = cp_sb.tile([65, 1024], BF16, tag="cp", name=f"cp{m}_{t4}")
                    nc.vector.tensor_copy(cp, po[0:65, :])
                    tails.append(gen_tail(m, t4, cp))

                # two chunks per pipeline step: scores x2 (one K=64 island),
                # exp x2, then the previous step's AVs x4 (one full-array
                # island) -- halves the K64<->K128 PE mode-switch penalty
                for c0 in range(0, NS, 2):
                    es = []
                    for c in (c0, c0 + 1):
                        psc = s_psum.tile([128, 1024], F32, tag="psc", name=f"ps{m}_{t4}_{c}")
                        nc.tensor.matmul(
                            psc[:, 0:512],
                            lhsT=kT[m][0:64, c * 128:(c + 1) * 128],
                            rhs=qT[m][0:64, sq],
                            start=True, stop=True,
                        )
                        nc.tensor.matmul(
                            psc[:, 512:1024],
                            lhsT=kT[m][64:128, c * 128:(c + 1) * 128],
                            rhs=qT[m][64:128, sq],
                            start=True, stop=True,
                        )
                        e = e_pool.tile([128, 1024], BF16, tag="e", name=f"e{m}_{t4}_{c}")
                        emit_exp(e, psc)
                        es.append(e)
                    # deferred AVs from the previous step (or the previous
                    # segment's last step + accumulator copy-out): their exps
                    # are done, so they never stall the PE queue
                    if pending[0] is not None:
                        pending[0]()
                        pending[0] = None
                    if first_seg:
                        # v(c0), v(c0+1) land ahead of their deferred AVs
                        for i in (c0, c0 + 1):
                            for _ in gen_v_proj(i):
                                pass
                    while sched and sched[0][0] <= c0 + 1:
                        sched.popleft()[1]()
                    if not first_seg and 1 <= c0 <= 13:
                        drain_tail_step(2 if len(tails) < 2 else 4)
                    if c0 < NS - 2:
                        def _p(es=es, c0=c0, emit_av=emit_av):
                            emit_av(c0, es[0])
                            emit_av(c0 + 1, es[1])
                        pending[0] = _p
                    else:
                        # retire the segment's last AVs inside the segment:
                        # the preceding v/filler PE work covers exp(14),(15)'s
                        # latency, and the next segment's first scores then
                        # enter a clean PE queue
                        emit_av(NS - 2, es[0])
                        emit_av(NS - 1, es[1])
                        finish_segment()

        if pending[0] is not None:
            pending[0]()
            pending[0] = None
        final[0] = True
        while tail_gen[0] is not None or tails:
            drain_tail_step(4)

    nc.compile()
    return nc


def _get_program():
    if "nc" not in _CACHE:
        _CACHE["nc"] = _build_program()
    return _CACHE["nc"]


def kernel(x, Wq, bq, Wk, bk, Wv, bv, _trace=False):
    bf = ml_dtypes.bfloat16
    x = np.asarray(x, dtype=np.float32)
    Wq = np.asarray(Wq, dtype=np.float32)
    Wk = np.asarray(Wk, dtype=np.float32)
    Wv = np.asarray(Wv, dtype=np.float32)
    bq = np.ascontiguousarray(np.asarray(bq, dtype=np.float32))
    bk = np.ascontiguousarray(np.asarray(bk, dtype=np.float32))
    bv = np.ascontiguousarray(np.asarray(bv, dtype=np.float32))

    nc = _get_program()

    in_maps = []
    for c in range(N_CORES):
        b, g = c // 2, c % 2
        cols = slice(g * DPC, (g + 1) * DPC)
        in_maps.append(
            {
                "xt": np.ascontiguousarray(x[b].T.astype(bf)),
                "wq": np.ascontiguousarray(Wq[:, cols].astype(bf)),
                "wk": np.ascontiguousarray(Wk[:, cols].astype(bf)),
                "wv": np.ascontiguousarray(Wv[:, cols].astype(bf)),
                "bq": np.ascontiguousarray(bq[cols]),
                "bk": np.ascontiguousarray(bk[cols]),
                "bv": np.ascontiguousarray(bv[cols]),
            }
        )

    res = run_bass_kernel_spmd(nc, in_maps, core_ids=list(range(N_CORES)), trace=_trace)
    _CACHE["last_results"] = res

    out = np.empty((B, S, D), dtype=np.float32)
    for c in range(N_CORES):
        b, g = c // 2, c % 2
        out[b, :, g * DPC:(g + 1) * DPC] = res.results[c]["out"]
    return out


# revision 19
# speedup vs baseline: 1.0932x; 1.0081x over previous
"""Multi-head attention (B=4, S=2048, D=1024, H=16, Dh=64) on 8 TRN2 NeuronCores.

Sharding: core c -> batch b = c//2, head-group g = c%2 (8 heads, output cols
g*512:(g+1)*512).  Host ships x pre-transposed ([D, S]) and weights in bf16;
each core runs attention for its (batch, 8 heads) slice; host concatenates the
per-core [2048, 512] outputs.

v3 structure (vs v2):
- DMA: ~12 fat dma_starts on 3 engine rings (sync/scalar/gpsimd) instead of
  81 small ones on 2 rings -- descriptor-gen (618ns per start, serialized
  per ring) stops gating the load; x lands in 4 column pieces so the pair-0
  projections start ~6us in.
- PE warms up on a zero dummy tile (no x dependency) so HAM hits 2.4GHz
  before the first projection.
- Scores per chunk: 2 concurrent K=64 matmuls (PE row groups 0-63/64-127),
  N=512 each, into separate PSUM banks of one [128,1024] tile.
- AV is software-pipelined one chunk late: AV(c) is emitted after exp(c+1),
  so it never heads-of-line-blocks the PE queue waiting on exp(c); the
  previous segment's last AV + accumulator copy-out ride the same mechanism.
- Projection filler is paced by explicit per-generator deadline windows
  (chunk index within the segment), balanced so no segment except (0,0)
  exceeds the exp-rate PE budget; v projections run inline in segment (0,0).
- Tails (transpose/normalize of finished segments) drain via a dynamic
  spreader, never at segment boundaries.
"""

import numpy as np
import ml_dtypes
from collections import deque
from contextlib import ExitStack

import concourse.bass as bass
import concourse.bacc as bacc
import concourse.mybir as mybir
import concourse.tile as tile
from concourse.bass_utils import run_bass_kernel_spmd
from concourse.masks import make_identity

F32 = mybir.dt.float32
BF16 = mybir.dt.bfloat16

B, S, D = 4, 2048, 1024
H, DH = 16, 64
N_CORES = 8
HPC = 8          # heads per core
DPC = HPC * DH   # output cols per core = 512
SCALE = 1.0 / 32.0  # 1/sqrt(D)

KD = D // 128    # 8 contraction chunks over d_in
NS = S // 128    # 16 sk chunks
MB = HPC // 2    # 4 head pairs
NT4 = S // 512   # 4 sq tiles of 512

_CACHE = {}


def _build_program():
    nc = bacc.Bacc("TRN2", target_bir_lowering=False, debug=False)

    # x ships pre-packed as 4 column-pieces of [p=128, j=8 K-chunks, 512 s]
    # (contiguous per piece) and the weights as [p, k, n] blobs: one DMA
    # descriptor then covers 8KB per partition instead of 1KB per (p, chunk)
    # row, making the load bandwidth-bound instead of descriptor-bound.
    xt_ext = nc.dram_tensor("xt", [NT4, 128, KD, 512], BF16, kind="ExternalInput").ap()
    wq_ext = nc.dram_tensor("wq", [128, KD, DPC], BF16, kind="ExternalInput").ap()
    wk_ext = nc.dram_tensor("wk", [128, KD, DPC], BF16, kind="ExternalInput").ap()
    wv_ext = nc.dram_tensor("wv", [128, KD, DPC], BF16, kind="ExternalInput").ap()
    bq_ext = nc.dram_tensor("bq", [DPC], F32, kind="ExternalInput").ap()
    bk_ext = nc.dram_tensor("bk", [DPC], F32, kind="ExternalInput").ap()
    bv_ext = nc.dram_tensor("bv", [DPC], F32, kind="ExternalInput").ap()
    out_ext = nc.dram_tensor("out", [S, DPC], F32, kind="ExternalOutput").ap()

    with tile.TileContext(nc, pool_alloc_mode="queue") as tc, ExitStack() as ctx:
        singles = ctx.enter_context(tc.tile_pool(name="singles", bufs=1))

        # --- biases: one consolidated DMA each (tiny, land instantly) ---
        bqm = singles.tile([128, MB], F32, tag="bqm")
        nc.sync.dma_start(out=bqm, in_=bq_ext.rearrange("(m p) -> p m", p=128))
        bkm = singles.tile([128, MB], F32, tag="bkm")
        nc.sync.dma_start(out=bkm, in_=bk_ext.rearrange("(m p) -> p m", p=128))
        bv_f32 = singles.tile([1, DPC], F32, tag="bv_f32")
        nc.sync.dma_start(out=bv_f32, in_=bv_ext.rearrange("(o n) -> o n", o=1))
        bv_row = singles.tile([1, DPC], BF16, tag="bv_row")
        nc.vector.tensor_copy(bv_row, bv_f32)

        xT = singles.tile([128, KD, S], BF16, tag="xT")
        wq_sb = singles.tile([128, KD, DPC], BF16, tag="wq_sb")
        wk_sb = singles.tile([128, KD, DPC], BF16, tag="wk_sb")
        wv_sb = singles.tile([128, KD, DPC], BF16, tag="wv_sb")

        # identity/memsets first so no engine's iota queues behind a fat
        # DMA-descriptor-generation instruction
        identity = singles.tile([128, 128], BF16, tag="identity")
        make_identity(nc, identity)
        ones_row = singles.tile([1, 128], BF16, tag="ones_row")
        nc.vector.memset(ones_row, 1.0)
        dummy = singles.tile([128, 512], BF16, tag="dummy")
        nc.vector.memset(dummy, 0.0)

        # Pre-packed blobs make each descriptor cover 8KB per partition, so
        # the load is bandwidth-bound, not descriptor-bound.
        # arrival waves: [x0 | wq | wk] -> [x1 | wv] -> [x2, x3]
        nc.sync.dma_start(out=xT[:, :, 0:512], in_=xt_ext[0])
        nc.sync.dma_start(out=wk_sb, in_=wk_ext)
        nc.sync.dma_start(out=xT[:, :, 512:1024], in_=xt_ext[1])
        nc.sync.dma_start(out=xT[:, :, 1024:1536], in_=xt_ext[2])
        nc.sync.dma_start(out=xT[:, :, 1536:2048], in_=xt_ext[3])
        # ring: scalar (idle until the first exp anyway)
        nc.scalar.dma_start(out=wq_sb, in_=wq_ext)
        nc.scalar.dma_start(out=wv_sb, in_=wv_ext)

        # --- persistent sbuf tensors: pair-packed qT/kT (rows 0-63 even
        # head's projection cols, 64-127 odd head's), v natural + ones col ---
        qT = [singles.tile([128, S], BF16, tag=f"qT{m}", name=f"qT{m}") for m in range(MB)]
        kT = [singles.tile([128, S], BF16, tag=f"kT{m}", name=f"kT{m}") for m in range(MB)]
        vsb = [singles.tile([128, HPC, DH + 1], BF16, tag=f"v{i}", name=f"v{i}") for i in range(NS)]
        out_full = [singles.tile([128, DPC], F32, tag=f"of{i}", name=f"of{i}") for i in range(NS)]

        # --- psum pools: scores 2x[128,1024]f32 (4 banks) + shared
        # accumulator/projection pool (4 banks) ---
        o_psum = ctx.enter_context(tc.tile_pool(name="o_psum", bufs=2, space="PSUM"))
        s_psum = ctx.enter_context(tc.tile_pool(name="s_psum", bufs=2, space="PSUM"))

        e_pool = ctx.enter_context(tc.tile_pool(name="e_pool", bufs=6))
        cp_sb = ctx.enter_context(tc.tile_pool(name="cp_sb", bufs=4))
        ot_sb = ctx.enter_context(tc.tile_pool(name="ot_sb", bufs=8))

        # warm the PE clock (HAM) on the zero dummy (no DMA dependency): the
        # SHORT window needs ~3.4us of sustained activity, 8 cold N=512 MMs
        warm = o_psum.tile([128, 512], F32, tag="po", name="warm")
        for _ in range(8):
            nc.tensor.matmul(warm, lhsT=identity, rhs=dummy, start=True, stop=True)

        # bv replicated across partitions (one K=1 matmul): the v bias is
        # applied once in the tail -- attention weights sum to 1 exactly, so
        # out = sum(attn (v'+bv)) = sum(attn v') + bv
        bvp = o_psum.tile([128, DPC], F32, tag="po", name="bvp")
        nc.tensor.matmul(bvp, lhsT=ones_row, rhs=bv_row, start=True, stop=True)
        bv_bc = singles.tile([128, DPC], BF16, tag="bv_bc")
        nc.vector.tensor_copy(bv_bc, bvp)

        def gen_q_proj(m, n):
            """q projection for pair m, seq cols n*512:(n+1)*512."""
            sl = slice(n * 512, (n + 1) * 512)
            ps = o_psum.tile([128, 512], F32, tag="po", name=f"ppq{m}_{n}")
            for k in range(KD):
                nc.tensor.matmul(
                    ps,
                    lhsT=wq_sb[:, k, m * 128:(m + 1) * 128],
                    rhs=xT[:, k, sl],
                    start=(k == 0),
                    stop=(k == KD - 1),
                )
                if k % 2 == 1:
                    yield
            nc.vector.tensor_scalar_add(qT[m][:, sl], ps, bqm[:, m:m + 1])

        def gen_k_proj(m, n):
            sl = slice(n * 512, (n + 1) * 512)
            ps = o_psum.tile([128, 512], F32, tag="po", name=f"ppk{m}_{n}")
            for k in range(KD):
                nc.tensor.matmul(
                    ps,
                    lhsT=wk_sb[:, k, m * 128:(m + 1) * 128],
                    rhs=xT[:, k, sl],
                    start=(k == 0),
                    stop=(k == KD - 1),
                )
                if k % 2 == 1:
                    yield
            nc.vector.tensor_scalar_add(kT[m][:, sl], ps, bkm[:, m:m + 1])

        def gen_v_proj(i):
            ps = o_psum.tile([128, 512], F32, tag="po", name=f"vp{i}")
            for k in range(KD):
                nc.tensor.matmul(
                    ps,
                    lhsT=xT[:, k, i * 128:(i + 1) * 128],
                    rhs=wv_sb[:, k, :],
                    start=(k == 0),
                    stop=(k == KD - 1),
                )
                if k % 2 == 1:
                    yield
            nc.vector.tensor_copy(
                vsb[i][:, :, 0:DH], ps.rearrange("p (h d) -> p h d", h=HPC)
            )
            nc.vector.memset(vsb[i][:, :, DH:DH + 1], 1.0)

        out_dma_n = [0]

        def emit_out_dma(i, tail=False):
            # sync ring during the attention stream; alternate sync/gpsimd
            # for the final drain (never scalar: it would steal exp time)
            eng = nc.gpsimd if (tail and out_dma_n[0] % 2) else nc.sync
            out_dma_n[0] += 1
            eng.dma_start(out=out_ext[i * 128:(i + 1) * 128, :], in_=out_full[i])

        def emit_exp(e, psc):
            # raw InstActivation with immediate bias/scale: skips the
            # per-partition bias-AP read the bass helper forces for Exp
            imm = lambda v: mybir.ImmediateValue(dtype=mybir.dt.float32, value=v)
            return nc.scalar.add_instruction(
                mybir.InstActivation(
                    name=nc.get_next_instruction_name(),
                    func=mybir.ActivationFunctionType.Exp,
                    ins=[nc.scalar.lower_ap(psc), imm(0.0), imm(SCALE), imm(0.0)],
                    outs=[nc.scalar.lower_ap(e)],
                )
            )

        def gen_tail(m, t4, cp):
            """Transpose/normalize pieces for segment (m, t4), reading the
            sbuf copy cp ([65,1024]: h0 | h1); one quantum per yield."""
            for c2 in range(4):
                for j in range(2):
                    pt = o_psum.tile([128, 65], BF16, tag="po", name=f"pt{m}_{t4}_{c2}_{j}")
                    nc.tensor.transpose(
                        pt,
                        cp[:, j * 512 + c2 * 128:j * 512 + (c2 + 1) * 128],
                        identity[0:65, 0:65],
                    )
                    ot = ot_sb.tile([128, 65], BF16, tag="ot", name=f"ot{m}_{t4}_{c2}_{j}")
                    nc.vector.tensor_copy(ot, pt)
                    rc = ot_sb.tile([128, 1], F32, tag="rc", name=f"rc{m}_{t4}_{c2}_{j}")
                    nc.vector.reciprocal(rc, ot[:, DH:DH + 1])
                    nc.vector.scalar_tensor_tensor(
                        out_full[t4 * 4 + c2][:, (2 * m + j) * DH:(2 * m + j + 1) * DH],
                        ot[:, 0:DH],
                        rc,
                        bv_bc[:, (2 * m + j) * DH:(2 * m + j + 1) * DH],
                        mybir.AluOpType.mult,
                        mybir.AluOpType.add,
                    )
                    if m == MB - 1 and j == 1:
                        emit_out_dma(t4 * 4 + c2, tail=final[0])
                    yield
            return

        # --- prelude: only q00/k00 gate the first chunk's scores ---
        for _ in gen_q_proj(0, 0):
            pass
        for _ in gen_k_proj(0, 0):
            pass

        # --- filler pacing: per-segment schedule of (due_chunk, fn), built
        # from explicit windows; tails drain via a dynamic spreader ---
        tails = []
        final = [False]
        tail_gen = [None, 0]  # generator, yields-left

        def drain_tail_step(k):
            for _ in range(k):
                if tail_gen[0] is None:
                    if not tails:
                        return
                    tail_gen[0] = tails.pop(0)
                    tail_gen[1] = 8
                try:
                    if tail_gen[1] > 0:
                        tail_gen[0].__next__()
                        tail_gen[1] -= 1
                    else:
                        deque(tail_gen[0], maxlen=0)
                        tail_gen[0] = None
                except StopIteration:
                    tail_gen[0] = None

        def make_sched(m, t4):
            sched = deque()

            def push(g, c0, c1, npops=4):
                total = npops + 1
                for i in range(total):
                    due = c0 + ((c1 - c0) * i) // (total - 1)
                    if i < npops:
                        sched.append((due, g.__next__))
                    else:
                        sched.append((due, lambda gg=g: deque(gg, maxlen=0)))

            # NOTE: every segment of pair m reads ALL of kT[m] (chunk c reads
            # sk cols c*128:(c+1)*128), so k(m, n) must complete inside the
            # FIRST segment of pair m by chunk 4n.  q(m, t4) is needed from
            # segment (m, t4) on.
            if t4 == 0:
                if m == 0:
                    push(gen_k_proj(0, 1), 0, 2)
                    push(gen_k_proj(0, 2), 3, 6)
                    push(gen_k_proj(0, 3), 7, 10)
                    push(gen_q_proj(0, 1), 11, 14)
                else:
                    push(gen_k_proj(m, 3), 1, 9)
                    push(gen_q_proj(m, 1), 10, 14)
            elif t4 == 1:
                if m == 0:
                    push(gen_q_proj(0, 2), 1, 8)
                else:
                    push(gen_q_proj(m, 2), 1, 7)
                    if m + 1 < MB:
                        push(gen_k_proj(m + 1, 1), 8, 15)
            elif t4 == 2:
                if m == 0:
                    push(gen_q_proj(0, 3), 1, 5)
                    push(gen_k_proj(1, 1), 6, 10)
                    push(gen_k_proj(1, 2), 11, 15)
                else:
                    push(gen_q_proj(m, 3), 1, 7)
                    if m + 1 < MB:
                        push(gen_k_proj(m + 1, 2), 8, 15)
            else:
                if m + 1 < MB:
                    push(gen_q_proj(m + 1, 0), 1, 6)
                    push(gen_k_proj(m + 1, 0), 7, 13)
            return sched

        pending = [None]

        for m in range(MB):
            for t4 in range(NT4):
                sched = make_sched(m, t4)
                po = o_psum.tile([128, 1024], F32, tag="po", name=f"po{m}_{t4}")
                sq = slice(t4 * 512, (t4 + 1) * 512)
                first_seg = (m == 0 and t4 == 0)

                def emit_av(c, e, po=po, m=m):
                    nc.tensor.matmul(
                        po[0:65, 0:512],
                        lhsT=vsb[c][:, 2 * m, :],
                        rhs=e[:, 0:512],
                        start=(c == 0), stop=(c == NS - 1),
                    )
                    nc.tensor.matmul(
                        po[0:65, 512:1024],
                        lhsT=vsb[c][:, 2 * m + 1, :],
                        rhs=e[:, 512:1024],
                        start=(c == 0), stop=(c == NS - 1),
                    )

                def finish_segment(po=po, m=m, t4=t4):
                    cp = cp_sb.tile([65, 1024], BF16, tag="cp", name=f"cp{m}_{t4}")
                    nc.vector.tensor_copy(cp, po[0:65, :])
                    tails.append(gen_tail(m, t4, cp))

                # two chunks per pipeline step: scores x2 (one K=64 island),
                # exp x2, then the previous step's AVs x4 (one full-array
                # island) -- halves the K64<->K128 PE mode-switch penalty
                for c0 in range(0, NS, 2):
                    es = []
                    for c in (c0, c0 + 1):
                        psc = s_psum.tile([128, 1024], F32, tag="psc", name=f"ps{m}_{t4}_{c}")
                        nc.tensor.matmul(
                            psc[:, 0:512],
                            lhsT=kT[m][0:64, c * 128:(c + 1) * 128],
                            rhs=qT[m][0:64, sq],
                            start=True, stop=True,
                        )
                        nc.tensor.matmul(
                            psc[:, 512:1024],
                            lhsT=kT[m][64:128, c * 128:(c + 1) * 128],
                            rhs=qT[m][64:128, sq],
                            start=True, stop=True,
                        )
                        e = e_pool.tile([128, 1024], BF16, tag="e", name=f"e{m}_{t4}_{c}")
                        emit_exp(e, psc)
                        es.append(e)
                    # deferred AVs from the previous step (or the previous
                    # segment's last step + accumulator copy-out): their exps
                    # are done, so they never stall the PE queue
                    if pending[0] is not None:
                        pending[0]()
                        pending[0] = None
                    if first_seg:
                        # v(c0), v(c0+1) land ahead of their deferred AVs
                        for i in (c0, c0 + 1):
                            for _ in gen_v_proj(i):
                                pass
                    while sched and sched[0][0] <= c0 + 1:
                        sched.popleft()[1]()
                    if not first_seg and 1 <= c0 <= 13:
                        drain_tail_step(2 if len(tails) < 2 else 4)
                    if c0 < NS - 2:
                        def _p(es=es, c0=c0, emit_av=emit_av):
                            emit_av(c0, es[0])
                            emit_av(c0 + 1, es[1])
                        pending[0] = _p
                    else:
                        # retire the segment's last AVs inside the segment:
                        # the preceding v/filler PE work covers exp(14),(15)'s
                        # latency, and the next segment's first scores then
                        # enter a clean PE queue
                        emit_av(NS - 2, es[0])
                        emit_av(NS - 1, es[1])
                        finish_segment()

        if pending[0] is not None:
            pending[0]()
            pending[0] = None
        final[0] = True
        while tail_gen[0] is not None or tails:
            drain_tail_step(4)

    nc.compile()
    return nc


def _get_program():
    if "nc" not in _CACHE:
        _CACHE["nc"] = _build_program()
    return _CACHE["nc"]


def kernel(x, Wq, bq, Wk, bk, Wv, bv, _trace=False):
    bf = ml_dtypes.bfloat16
    x = np.asarray(x, dtype=np.float32)
    Wq = np.asarray(Wq, dtype=np.float32)
    Wk = np.asarray(Wk, dtype=np.float32)
    Wv = np.asarray(Wv, dtype=np.float32)
    bq = np.ascontiguousarray(np.asarray(bq, dtype=np.float32))
    bk = np.ascontiguousarray(np.asarray(bk, dtype=np.float32))
    bv = np.ascontiguousarray(np.asarray(bv, dtype=np.float32))

    nc = _get_program()

    def pack_w(W):
        # [D, DPC] -> [p=128, k=8, n] blob (8KB contiguous per partition)
        return np.ascontiguousarray(W.astype(bf).reshape(KD, 128, DPC).transpose(1, 0, 2))

    in_maps = []
    for c in range(N_CORES):
        b, g = c // 2, c % 2
        cols = slice(g * DPC, (g + 1) * DPC)
        # x[b].T is [D, S]; pack into 4 column-pieces of [p, j, 512]
        xt = x[b].T.astype(bf).reshape(KD, 128, NT4, 512)
        xt = np.ascontiguousarray(xt.transpose(2, 1, 0, 3))
        in_maps.append(
            {
                "xt": xt,
                "wq": pack_w(Wq[:, cols]),
                "wk": pack_w(Wk[:, cols]),
                "wv": pack_w(Wv[:, cols]),
                "bq": np.ascontiguousarray(bq[cols]),
                "bk": np.ascontiguousarray(bk[cols]),
                "bv": np.ascontiguousarray(bv[cols]),
            }
        )

    res = run_bass_kernel_spmd(nc, in_maps, core_ids=list(range(N_CORES)), trace=_trace)
    _CACHE["last_results"] = res

    out = np.empty((B, S, D), dtype=np.float32)
    for c in range(N_CORES):
        b, g = c // 2, c % 2
        out[b, :, g * DPC:(g + 1) * DPC] = res.results[c]["out"]
    return out


# revision 21
# speedup vs baseline: 1.0997x; 1.0059x over previous
"""Multi-head attention (B=4, S=2048, D=1024, H=16, Dh=64) on 8 TRN2 NeuronCores.

Sharding: core c -> batch b = c//2, head-group g = c%2 (8 heads, output cols
g*512:(g+1)*512).  Host ships x pre-transposed ([D, S]) and weights in bf16;
each core runs attention for its (batch, 8 heads) slice; host concatenates the
per-core [2048, 512] outputs.

v3 structure (vs v2):
- DMA: ~12 fat dma_starts on 3 engine rings (sync/scalar/gpsimd) instead of
  81 small ones on 2 rings -- descriptor-gen (618ns per start, serialized
  per ring) stops gating the load; x lands in 4 column pieces so the pair-0
  projections start ~6us in.
- PE warms up on a zero dummy tile (no x dependency) so HAM hits 2.4GHz
  before the first projection.
- Scores per chunk: 2 concurrent K=64 matmuls (PE row groups 0-63/64-127),
  N=512 each, into separate PSUM banks of one [128,1024] tile.
- AV is software-pipelined one chunk late: AV(c) is emitted after exp(c+1),
  so it never heads-of-line-blocks the PE queue waiting on exp(c); the
  previous segment's last AV + accumulator copy-out ride the same mechanism.
- Projection filler is paced by explicit per-generator deadline windows
  (chunk index within the segment), balanced so no segment except (0,0)
  exceeds the exp-rate PE budget; v projections run inline in segment (0,0).
- Tails (transpose/normalize of finished segments) drain via a dynamic
  spreader, never at segment boundaries.
"""

import numpy as np
import ml_dtypes
from collections import deque
from contextlib import ExitStack

import concourse.bass as bass
import concourse.bacc as bacc
import concourse.mybir as mybir
import concourse.tile as tile
from concourse.bass_utils import run_bass_kernel_spmd
from concourse.masks import make_identity

F32 = mybir.dt.float32
BF16 = mybir.dt.bfloat16

B, S, D = 4, 2048, 1024
H, DH = 16, 64
N_CORES = 8
HPC = 8          # heads per core
DPC = HPC * DH   # output cols per core = 512
SCALE = 1.0 / 32.0  # 1/sqrt(D)

KD = D // 128    # 8 contraction chunks over d_in
NS = S // 128    # 16 sk chunks
MB = HPC // 2    # 4 head pairs
NT4 = S // 512   # 4 sq tiles of 512

_CACHE = {}


def _build_program():
    nc = bacc.Bacc("TRN2", target_bir_lowering=False, debug=False)

    # x ships pre-packed as 4 column-pieces of [p=128, j=8 K-chunks, 512 s]
    # (contiguous per piece) and the weights as [p, k, n] blobs: one DMA
    # descriptor then covers 8KB per partition instead of 1KB per (p, chunk)
    # row, making the load bandwidth-bound instead of descriptor-bound.
    xt_ext = nc.dram_tensor("xt", [NT4, 128, KD, 512], BF16, kind="ExternalInput").ap()
    wq_ext = nc.dram_tensor("wq", [128, KD, DPC], BF16, kind="ExternalInput").ap()
    wk_ext = nc.dram_tensor("wk", [128, KD, DPC], BF16, kind="ExternalInput").ap()
    wv_ext = nc.dram_tensor("wv", [128, KD, DPC], BF16, kind="ExternalInput").ap()
    bq_ext = nc.dram_tensor("bq", [DPC], F32, kind="ExternalInput").ap()
    bk_ext = nc.dram_tensor("bk", [DPC], F32, kind="ExternalInput").ap()
    bv_ext = nc.dram_tensor("bv", [DPC], F32, kind="ExternalInput").ap()
    out_ext = nc.dram_tensor("out", [S, DPC], F32, kind="ExternalOutput").ap()

    with tile.TileContext(nc, pool_alloc_mode="queue") as tc, ExitStack() as ctx:
        singles = ctx.enter_context(tc.tile_pool(name="singles", bufs=1))

        # --- biases: one consolidated DMA each (tiny, land instantly) ---
        bqm = singles.tile([128, MB], F32, tag="bqm")
        nc.sync.dma_start(out=bqm, in_=bq_ext.rearrange("(m p) -> p m", p=128))
        bkm = singles.tile([128, MB], F32, tag="bkm")
        nc.sync.dma_start(out=bkm, in_=bk_ext.rearrange("(m p) -> p m", p=128))
        bv_f32 = singles.tile([1, DPC], F32, tag="bv_f32")
        nc.sync.dma_start(out=bv_f32, in_=bv_ext.rearrange("(o n) -> o n", o=1))
        bv_row = singles.tile([1, DPC], BF16, tag="bv_row")
        nc.vector.tensor_copy(bv_row, bv_f32)

        xT = singles.tile([128, KD, S], BF16, tag="xT")
        wq_sb = singles.tile([128, KD, DPC], BF16, tag="wq_sb")
        wk_sb = singles.tile([128, KD, DPC], BF16, tag="wk_sb")
        wv_sb = singles.tile([128, KD, DPC], BF16, tag="wv_sb")

        # identity/memsets first so no engine's iota queues behind a fat
        # DMA-descriptor-generation instruction
        identity = singles.tile([128, 128], BF16, tag="identity")
        make_identity(nc, identity)
        ones_row = singles.tile([1, 128], BF16, tag="ones_row")
        nc.vector.memset(ones_row, 1.0)
        dummy = singles.tile([128, 512], BF16, tag="dummy")
        nc.vector.memset(dummy, 0.0)

        # Pre-packed blobs make each descriptor cover 8KB per partition, so
        # the load is bandwidth-bound, not descriptor-bound.
        # arrival waves: [wq | x0] -> [wk | wv] -> [x1, x2, x3]
        nc.sync.dma_start(out=wq_sb, in_=wq_ext)
        nc.sync.dma_start(out=wk_sb, in_=wk_ext)
        nc.sync.dma_start(out=xT[:, :, 512:1024], in_=xt_ext[1])
        nc.sync.dma_start(out=xT[:, :, 1024:1536], in_=xt_ext[2])
        nc.sync.dma_start(out=xT[:, :, 1536:2048], in_=xt_ext[3])
        # ring: scalar (idle until the first exp anyway)
        nc.scalar.dma_start(out=xT[:, :, 0:512], in_=xt_ext[0])
        nc.scalar.dma_start(out=wv_sb, in_=wv_ext)

        # --- persistent sbuf tensors: pair-packed qT/kT (rows 0-63 even
        # head's projection cols, 64-127 odd head's), v natural + ones col ---
        qT = [singles.tile([128, S], BF16, tag=f"qT{m}", name=f"qT{m}") for m in range(MB)]
        kT = [singles.tile([128, S], BF16, tag=f"kT{m}", name=f"kT{m}") for m in range(MB)]
        vsb = [singles.tile([128, HPC, DH + 1], BF16, tag=f"v{i}", name=f"v{i}") for i in range(NS)]
        out_full = [singles.tile([128, DPC], F32, tag=f"of{i}", name=f"of{i}") for i in range(NS)]

        # --- psum pools: scores 2x[128,1024]f32 (4 banks) + shared
        # accumulator/projection pool (4 banks) ---
        o_psum = ctx.enter_context(tc.tile_pool(name="o_psum", bufs=2, space="PSUM"))
        s_psum = ctx.enter_context(tc.tile_pool(name="s_psum", bufs=2, space="PSUM"))

        e_pool = ctx.enter_context(tc.tile_pool(name="e_pool", bufs=6))
        cp_sb = ctx.enter_context(tc.tile_pool(name="cp_sb", bufs=4))
        ot_sb = ctx.enter_context(tc.tile_pool(name="ot_sb", bufs=8))

        # warm the PE clock (HAM) on the zero dummy (no DMA dependency): 8
        # cold pulses flip the SHORT window to 2.4GHz, 8 more bridge most of
        # the remaining DMA wait so q00 starts warm
        warm = o_psum.tile([128, 512], F32, tag="po", name="warm")
        for _ in range(16):
            nc.tensor.matmul(warm, lhsT=identity, rhs=dummy, start=True, stop=True)

        # bv replicated across partitions (one K=1 matmul): the v bias is
        # applied once in the tail -- attention weights sum to 1 exactly, so
        # out = sum(attn (v'+bv)) = sum(attn v') + bv
        bvp = o_psum.tile([128, DPC], F32, tag="po", name="bvp")
        nc.tensor.matmul(bvp, lhsT=ones_row, rhs=bv_row, start=True, stop=True)
        bv_bc = singles.tile([128, DPC], BF16, tag="bv_bc")
        nc.vector.tensor_copy(bv_bc, bvp)

        def gen_q_proj(m, n):
            """q projection for pair m, seq cols n*512:(n+1)*512."""
            sl = slice(n * 512, (n + 1) * 512)
            ps = o_psum.tile([128, 512], F32, tag="po", name=f"ppq{m}_{n}")
            for k in range(KD):
                nc.tensor.matmul(
                    ps,
                    lhsT=wq_sb[:, k, m * 128:(m + 1) * 128],
                    rhs=xT[:, k, sl],
                    start=(k == 0),
                    stop=(k == KD - 1),
                )
                if k % 2 == 1:
                    yield
            nc.vector.tensor_scalar_add(qT[m][:, sl], ps, bqm[:, m:m + 1])

        def gen_k_proj(m, n):
            sl = slice(n * 512, (n + 1) * 512)
            ps = o_psum.tile([128, 512], F32, tag="po", name=f"ppk{m}_{n}")
            for k in range(KD):
                nc.tensor.matmul(
                    ps,
                    lhsT=wk_sb[:, k, m * 128:(m + 1) * 128],
                    rhs=xT[:, k, sl],
                    start=(k == 0),
                    stop=(k == KD - 1),
                )
                if k % 2 == 1:
                    yield
            nc.vector.tensor_scalar_add(kT[m][:, sl], ps, bkm[:, m:m + 1])

        def gen_v_proj(i):
            ps = o_psum.tile([128, 512], F32, tag="po", name=f"vp{i}")
            for k in range(KD):
                nc.tensor.matmul(
                    ps,
                    lhsT=xT[:, k, i * 128:(i + 1) * 128],
                    rhs=wv_sb[:, k, :],
                    start=(k == 0),
                    stop=(k == KD - 1),
                )
                if k % 2 == 1:
                    yield
            nc.vector.tensor_copy(
                vsb[i][:, :, 0:DH], ps.rearrange("p (h d) -> p h d", h=HPC)
            )
            nc.vector.memset(vsb[i][:, :, DH:DH + 1], 1.0)

        out_dma_n = [0]

        def emit_out_dma(i, tail=False):
            # sync ring during the attention stream; alternate sync/gpsimd
            # for the final drain (never scalar: it would steal exp time)
            eng = nc.gpsimd if (tail and out_dma_n[0] % 2) else nc.sync
            out_dma_n[0] += 1
            eng.dma_start(out=out_ext[i * 128:(i + 1) * 128, :], in_=out_full[i])

        def emit_exp(e, psc):
            # raw InstActivation with immediate bias/scale: skips the
            # per-partition bias-AP read the bass helper forces for Exp
            imm = lambda v: mybir.ImmediateValue(dtype=mybir.dt.float32, value=v)
            return nc.scalar.add_instruction(
                mybir.InstActivation(
                    name=nc.get_next_instruction_name(),
                    func=mybir.ActivationFunctionType.Exp,
                    ins=[nc.scalar.lower_ap(psc), imm(0.0), imm(SCALE), imm(0.0)],
                    outs=[nc.scalar.lower_ap(e)],
                )
            )

        def gen_tail(m, t4, cp):
            """Transpose/normalize pieces for segment (m, t4), reading the
            sbuf copy cp ([65,1024]: h0 | h1); one quantum per yield."""
            for c2 in range(4):
                for j in range(2):
                    pt = o_psum.tile([128, 65], BF16, tag="po", name=f"pt{m}_{t4}_{c2}_{j}")
                    nc.tensor.transpose(
                        pt,
                        cp[:, j * 512 + c2 * 128:j * 512 + (c2 + 1) * 128],
                        identity[0:65, 0:65],
                    )
                    ot = ot_sb.tile([128, 65], BF16, tag="ot", name=f"ot{m}_{t4}_{c2}_{j}")
                    nc.vector.tensor_copy(ot, pt)
                    rc = ot_sb.tile([128, 1], F32, tag="rc", name=f"rc{m}_{t4}_{c2}_{j}")
                    nc.vector.reciprocal(rc, ot[:, DH:DH + 1])
                    nc.vector.scalar_tensor_tensor(
                        out_full[t4 * 4 + c2][:, (2 * m + j) * DH:(2 * m + j + 1) * DH],
                        ot[:, 0:DH],
                        rc,
                        bv_bc[:, (2 * m + j) * DH:(2 * m + j + 1) * DH],
                        mybir.AluOpType.mult,
                        mybir.AluOpType.add,
                    )
                    if m == MB - 1 and j == 1:
                        emit_out_dma(t4 * 4 + c2, tail=final[0])
                    yield
            return

        # --- prelude: only q00/k00 gate the first chunk's scores ---
        for _ in gen_q_proj(0, 0):
            pass
        for _ in gen_k_proj(0, 0):
            pass

        # --- filler pacing: per-segment schedule of (due_chunk, fn), built
        # from explicit windows; tails drain via a dynamic spreader ---
        tails = []
        final = [False]
        tail_gen = [None, 0]  # generator, yields-left

        def drain_tail_step(k):
            for _ in range(k):
                if tail_gen[0] is None:
                    if not tails:
                        return
                    tail_gen[0] = tails.pop(0)
                    tail_gen[1] = 8
                try:
                    if tail_gen[1] > 0:
                        tail_gen[0].__next__()
                        tail_gen[1] -= 1
                    else:
                        deque(tail_gen[0], maxlen=0)
                        tail_gen[0] = None
                except StopIteration:
                    tail_gen[0] = None

        def make_sched(m, t4):
            sched = deque()

            def push(g, c0, c1, npops=4):
                total = npops + 1
                for i in range(total):
                    due = c0 + ((c1 - c0) * i) // (total - 1)
                    if i < npops:
                        sched.append((due, g.__next__))
                    else:
                        sched.append((due, lambda gg=g: deque(gg, maxlen=0)))

            # NOTE: every segment of pair m reads ALL of kT[m] (chunk c reads
            # sk cols c*128:(c+1)*128), so k(m, n) must complete inside the
            # FIRST segment of pair m by chunk 4n.  q(m, t4) is needed from
            # segment (m, t4) on.
            if t4 == 0:
                if m == 0:
                    push(gen_k_proj(0, 1), 0, 2)
                    push(gen_k_proj(0, 2), 3, 6)
                    push(gen_k_proj(0, 3), 7, 10)
                    push(gen_q_proj(0, 1), 11, 14)
                else:
                    push(gen_k_proj(m, 3), 1, 9)
                    push(gen_q_proj(m, 1), 10, 14)
            elif t4 == 1:
                if m == 0:
                    push(gen_q_proj(0, 2), 1, 8)
                else:
                    push(gen_q_proj(m, 2), 1, 7)
                    if m + 1 < MB:
                        push(gen_k_proj(m + 1, 1), 8, 15)
            elif t4 == 2:
                if m == 0:
                    push(gen_q_proj(0, 3), 1, 5)
                    push(gen_k_proj(1, 1), 6, 10)
                    push(gen_k_proj(1, 2), 11, 15)
                else:
                    push(gen_q_proj(m, 3), 1, 7)
                    if m + 1 < MB:
                        push(gen_k_proj(m + 1, 2), 8, 15)
            else:
                if m + 1 < MB:
                    push(gen_q_proj(m + 1, 0), 1, 6)
                    push(gen_k_proj(m + 1, 0), 7, 13)
            return sched

        pending = [None]

        for m in range(MB):
            for t4 in range(NT4):
                sched = make_sched(m, t4)
                po = o_psum.tile([128, 1024], F32, tag="po", name=f"po{m}_{t4}")
                sq = slice(t4 * 512, (t4 + 1) * 512)
                first_seg = (m == 0 and t4 == 0)

                def emit_av(c, e, po=po, m=m):
                    nc.tensor.matmul(
                        po[0:65, 0:512],
                        lhsT=vsb[c][:, 2 * m, :],
                        rhs=e[:, 0:512],
                        start=(c == 0), stop=(c == NS - 1),
                    )
                    nc.tensor.matmul(
                        po[0:65, 512:1024],
                        lhsT=vsb[c][:, 2 * m + 1, :],
                        rhs=e[:, 512:1024],
                        start=(c == 0), stop=(c == NS - 1),
                    )

                def finish_segment(po=po, m=m, t4=t4):
                    cp = cp_sb.tile([65, 1024], BF16, tag="cp", name=f"cp{m}_{t4}")
                    nc.vector.tensor_copy(cp, po[0:65, :])
                    tails.append(gen_tail(m, t4, cp))

                # two chunks per pipeline step: scores x2 (one K=64 island),
                # exp x2, then the previous step's AVs x4 (one full-array
                # island) -- halves the K64<->K128 PE mode-switch penalty
                for c0 in range(0, NS, 2):
                    es = []
                    for c in (c0, c0 + 1):
                        psc = s_psum.tile([128, 1024], F32, tag="psc", name=f"ps{m}_{t4}_{c}")
                        nc.tensor.matmul(
                            psc[:, 0:512],
                            lhsT=kT[m][0:64, c * 128:(c + 1) * 128],
                            rhs=qT[m][0:64, sq],
                            start=True, stop=True,
                        )
                        nc.tensor.matmul(
                            psc[:, 512:1024],
                            lhsT=kT[m][64:128, c * 128:(c + 1) * 128],
                            rhs=qT[m][64:128, sq],
                            start=True, stop=True,
                        )
                        e = e_pool.tile([128, 1024], BF16, tag="e", name=f"e{m}_{t4}_{c}")
                        emit_exp(e, psc)
                        es.append(e)
                    # deferred AVs from the previous step (or the previous
                    # segment's last step + accumulator copy-out): their exps
                    # are done, so they never stall the PE queue
                    if pending[0] is not None:
                        pending[0]()
                        pending[0] = None
                    if first_seg:
                        # v(c0), v(c0+1) land ahead of their deferred AVs
                        for i in (c0, c0 + 1):
                            for _ in gen_v_proj(i):
                                pass
                    while sched and sched[0][0] <= c0 + 1:
                        sched.popleft()[1]()
                    if not first_seg and 1 <= c0 <= 13:
                        drain_tail_step(2 if len(tails) < 2 else 4)
                    if c0 < NS - 2:
                        def _p(es=es, c0=c0, emit_av=emit_av):
                            emit_av(c0, es[0])
                            emit_av(c0 + 1, es[1])
                        pending[0] = _p
                    else:
                        # retire the segment's last AVs inside the segment:
                        # the preceding v/filler PE work covers exp(14),(15)'s
                        # latency, and the next segment's first scores then
                        # enter a clean PE queue
                        emit_av(NS - 2, es[0])
                        emit_av(NS - 1, es[1])
                        finish_segment()

        if pending[0] is not None:
            pending[0]()
            pending[0] = None
        final[0] = True
        while tail_gen[0] is not None or tails:
            drain_tail_step(4)

    nc.compile()
    return nc


def _get_program():
    if "nc" not in _CACHE:
        _CACHE["nc"] = _build_program()
    return _CACHE["nc"]


def kernel(x, Wq, bq, Wk, bk, Wv, bv, _trace=False):
    bf = ml_dtypes.bfloat16
    x = np.asarray(x, dtype=np.float32)
    Wq = np.asarray(Wq, dtype=np.float32)
    Wk = np.asarray(Wk, dtype=np.float32)
    Wv = np.asarray(Wv, dtype=np.float32)
    bq = np.ascontiguousarray(np.asarray(bq, dtype=np.float32))
    bk = np.ascontiguousarray(np.asarray(bk, dtype=np.float32))
    bv = np.ascontiguousarray(np.asarray(bv, dtype=np.float32))

    nc = _get_program()

    def pack_w(W):
        # [D, DPC] -> [p=128, k=8, n] blob (8KB contiguous per partition)
        return np.ascontiguousarray(W.astype(bf).reshape(KD, 128, DPC).transpose(1, 0, 2))

    in_maps = []
    for c in range(N_CORES):
        b, g = c // 2, c % 2
        cols = slice(g * DPC, (g + 1) * DPC)
        # x[b].T is [D, S]; pack into 4 column-pieces of [p, j, 512]
        xt = x[b].T.astype(bf).reshape(KD, 128, NT4, 512)
        xt = np.ascontiguousarray(xt.transpose(2, 1, 0, 3))
        in_maps.append(
            {
                "xt": xt,
                "wq": pack_w(Wq[:, cols]),
                "wk": pack_w(Wk[:, cols]),
                "wv": pack_w(Wv[:, cols]),
                "bq": np.ascontiguousarray(bq[cols]),
                "bk": np.ascontiguousarray(bk[cols]),
                "bv": np.ascontiguousarray(bv[cols]),
            }
        )

    res = run_bass_kernel_spmd(nc, in_maps, core_ids=list(range(N_CORES)), trace=_trace)
    _CACHE["last_results"] = res

    out = np.empty((B, S, D), dtype=np.float32)
    for c in range(N_CORES):
        b, g = c // 2, c % 2
        out[b, :, g * DPC:(g + 1) * DPC] = res.results[c]["out"]
    return out


# revision 24
# speedup vs baseline: 1.1070x; 1.0067x over previous
"""Multi-head attention (B=4, S=2048, D=1024, H=16, Dh=64) on 8 TRN2 NeuronCores.

Sharding: core c -> batch b = c//2, head-group g = c%2 (8 heads, output cols
g*512:(g+1)*512).  Host ships x pre-transposed ([D, S]) and weights in bf16;
each core runs attention for its (batch, 8 heads) slice; host concatenates the
per-core [2048, 512] outputs.

v3 structure (vs v2):
- DMA: ~12 fat dma_starts on 3 engine rings (sync/scalar/gpsimd) instead of
  81 small ones on 2 rings -- descriptor-gen (618ns per start, serialized
  per ring) stops gating the load; x lands in 4 column pieces so the pair-0
  projections start ~6us in.
- PE warms up on a zero dummy tile (no x dependency) so HAM hits 2.4GHz
  before the first projection.
- Scores per chunk: 2 concurrent K=64 matmuls (PE row groups 0-63/64-127),
  N=512 each, into separate PSUM banks of one [128,1024] tile.
- AV is software-pipelined one chunk late: AV(c) is emitted after exp(c+1),
  so it never heads-of-line-blocks the PE queue waiting on exp(c); the
  previous segment's last AV + accumulator copy-out ride the same mechanism.
- Projection filler is paced by explicit per-generator deadline windows
  (chunk index within the segment), balanced so no segment except (0,0)
  exceeds the exp-rate PE budget; v projections run inline in segment (0,0).
- Tails (transpose/normalize of finished segments) drain via a dynamic
  spreader, never at segment boundaries.
"""

import numpy as np
import ml_dtypes
from collections import deque
from contextlib import ExitStack

import concourse.bass as bass
import concourse.bacc as bacc
import concourse.mybir as mybir
import concourse.tile as tile
from concourse.bass_utils import run_bass_kernel_spmd
from concourse.masks import make_identity

F32 = mybir.dt.float32
BF16 = mybir.dt.bfloat16

B, S, D = 4, 2048, 1024
H, DH = 16, 64
N_CORES = 8
HPC = 8          # heads per core
DPC = HPC * DH   # output cols per core = 512
SCALE = 1.0 / 32.0  # 1/sqrt(D)

KD = D // 128    # 8 contraction chunks over d_in
NS = S // 128    # 16 sk chunks
MB = HPC // 2    # 4 head pairs
NT4 = S // 512   # 4 sq tiles of 512

_CACHE = {}


def _build_program():
    nc = bacc.Bacc("TRN2", target_bir_lowering=False, debug=False)

    # x ships pre-packed as 4 column-pieces of [p=128, j=8 K-chunks, 512 s]
    # (contiguous per piece) and the weights as [p, k, n] blobs: one DMA
    # descriptor then covers 8KB per partition instead of 1KB per (p, chunk)
    # row, making the load bandwidth-bound instead of descriptor-bound.
    xt_ext = nc.dram_tensor("xt", [NT4, 128, KD, 512], BF16, kind="ExternalInput").ap()
    wq_ext = nc.dram_tensor("wq", [128, KD, DPC], BF16, kind="ExternalInput").ap()
    wk_ext = nc.dram_tensor("wk", [128, KD, DPC], BF16, kind="ExternalInput").ap()
    wv_ext = nc.dram_tensor("wv", [128, KD, DPC], BF16, kind="ExternalInput").ap()
    bq_ext = nc.dram_tensor("bq", [DPC], F32, kind="ExternalInput").ap()
    bk_ext = nc.dram_tensor("bk", [DPC], F32, kind="ExternalInput").ap()
    bv_ext = nc.dram_tensor("bv", [DPC], F32, kind="ExternalInput").ap()
    out_ext = nc.dram_tensor("out", [S, DPC], F32, kind="ExternalOutput").ap()

    with tile.TileContext(nc, pool_alloc_mode="queue") as tc, ExitStack() as ctx:
        singles = ctx.enter_context(tc.tile_pool(name="singles", bufs=1))

        # --- biases: one consolidated DMA each (tiny, land instantly) ---
        bqm = singles.tile([128, MB], F32, tag="bqm")
        nc.sync.dma_start(out=bqm, in_=bq_ext.rearrange("(m p) -> p m", p=128))
        bkm = singles.tile([128, MB], F32, tag="bkm")
        nc.sync.dma_start(out=bkm, in_=bk_ext.rearrange("(m p) -> p m", p=128))
        bv_f32 = singles.tile([1, DPC], F32, tag="bv_f32")
        nc.sync.dma_start(out=bv_f32, in_=bv_ext.rearrange("(o n) -> o n", o=1))
        bv_row = singles.tile([1, DPC], BF16, tag="bv_row")
        nc.vector.tensor_copy(bv_row, bv_f32)

        xT = singles.tile([128, KD, S], BF16, tag="xT")
        wq_sb = singles.tile([128, KD, DPC], BF16, tag="wq_sb")
        wk_sb = singles.tile([128, KD, DPC], BF16, tag="wk_sb")
        wv_sb = singles.tile([128, KD, DPC], BF16, tag="wv_sb")

        # identity/memsets first so no engine's iota queues behind a fat
        # DMA-descriptor-generation instruction
        identity = singles.tile([128, 128], BF16, tag="identity")
        make_identity(nc, identity)
        ones_row = singles.tile([1, 128], BF16, tag="ones_row")
        nc.vector.memset(ones_row, 1.0)
        dummy = singles.tile([128, 512], BF16, tag="dummy")
        nc.vector.memset(dummy, 0.0)

        # Pre-packed blobs make each descriptor cover 8KB per partition, so
        # the load is bandwidth-bound, not descriptor-bound.
        # arrival waves: [wq | x0] -> [wk | wv] -> [x2, x3 | x1]
        nc.sync.dma_start(out=wq_sb, in_=wq_ext)
        nc.sync.dma_start(out=wk_sb, in_=wk_ext)
        nc.sync.dma_start(out=xT[:, :, 1024:1536], in_=xt_ext[2])
        nc.sync.dma_start(out=xT[:, :, 1536:2048], in_=xt_ext[3])
        # ring: scalar (idle until the first exp anyway)
        nc.scalar.dma_start(out=xT[:, :, 0:512], in_=xt_ext[0])
        nc.scalar.dma_start(out=wv_sb, in_=wv_ext)
        nc.scalar.dma_start(out=xT[:, :, 512:1024], in_=xt_ext[1])

        # --- persistent sbuf tensors: pair-packed qT/kT (rows 0-63 even
        # head's projection cols, 64-127 odd head's), v natural + ones col ---
        qT = [singles.tile([128, S], BF16, tag=f"qT{m}", name=f"qT{m}") for m in range(MB)]
        kT = [singles.tile([128, S], BF16, tag=f"kT{m}", name=f"kT{m}") for m in range(MB)]
        vsb = [singles.tile([128, HPC, DH + 1], BF16, tag=f"v{i}", name=f"v{i}") for i in range(NS)]
        out_full = [singles.tile([128, DPC], F32, tag=f"of{i}", name=f"of{i}") for i in range(NS)]

        # --- psum pools: scores 2x[128,1024]f32 (4 banks) + shared
        # accumulator/projection pool (4 banks) ---
        o_psum = ctx.enter_context(tc.tile_pool(name="o_psum", bufs=2, space="PSUM"))
        s_psum = ctx.enter_context(tc.tile_pool(name="s_psum", bufs=2, space="PSUM"))

        e_pool = ctx.enter_context(tc.tile_pool(name="e_pool", bufs=6))
        cp_sb = ctx.enter_context(tc.tile_pool(name="cp_sb", bufs=4))
        ot_sb = ctx.enter_context(tc.tile_pool(name="ot_sb", bufs=8))

        # warm the PE clock (HAM) on the zero dummy (no DMA dependency): 8
        # cold pulses flip the SHORT window to 2.4GHz, 8 more bridge most of
        # the remaining DMA wait so q00 starts warm
        warm = o_psum.tile([128, 512], F32, tag="po", name="warm")
        for _ in range(16):
            nc.tensor.matmul(warm, lhsT=identity, rhs=dummy, start=True, stop=True)

        # bv replicated across partitions (one K=1 matmul): the v bias is
        # applied once in the tail -- attention weights sum to 1 exactly, so
        # out = sum(attn (v'+bv)) = sum(attn v') + bv
        bvp = o_psum.tile([128, DPC], F32, tag="po", name="bvp")
        nc.tensor.matmul(bvp, lhsT=ones_row, rhs=bv_row, start=True, stop=True)
        bv_bc = singles.tile([128, DPC], BF16, tag="bv_bc")
        nc.vector.tensor_copy(bv_bc, bvp)

        def gen_q_proj(m, n):
            """q projection for pair m, seq cols n*512:(n+1)*512."""
            sl = slice(n * 512, (n + 1) * 512)
            ps = o_psum.tile([128, 512], F32, tag="po", name=f"ppq{m}_{n}")
            for k in range(KD):
                nc.tensor.matmul(
                    ps,
                    lhsT=wq_sb[:, k, m * 128:(m + 1) * 128],
                    rhs=xT[:, k, sl],
                    start=(k == 0),
                    stop=(k == KD - 1),
                )
                if k % 2 == 1:
                    yield
            nc.vector.tensor_scalar_add(qT[m][:, sl], ps, bqm[:, m:m + 1])

        def gen_k_proj(m, n):
            sl = slice(n * 512, (n + 1) * 512)
            ps = o_psum.tile([128, 512], F32, tag="po", name=f"ppk{m}_{n}")
            for k in range(KD):
                nc.tensor.matmul(
                    ps,
                    lhsT=wk_sb[:, k, m * 128:(m + 1) * 128],
                    rhs=xT[:, k, sl],
                    start=(k == 0),
                    stop=(k == KD - 1),
                )
                if k % 2 == 1:
                    yield
            nc.vector.tensor_scalar_add(kT[m][:, sl], ps, bkm[:, m:m + 1])

        def gen_v_proj(i):
            ps = o_psum.tile([128, 512], F32, tag="po", name=f"vp{i}")
            for k in range(KD):
                nc.tensor.matmul(
                    ps,
                    lhsT=xT[:, k, i * 128:(i + 1) * 128],
                    rhs=wv_sb[:, k, :],
                    start=(k == 0),
                    stop=(k == KD - 1),
                )
                if k % 2 == 1:
                    yield
            nc.vector.tensor_copy(
                vsb[i][:, :, 0:DH], ps.rearrange("p (h d) -> p h d", h=HPC)
            )
            nc.vector.memset(vsb[i][:, :, DH:DH + 1], 1.0)

        out_dma_n = [0]

        def emit_out_dma(i, tail=False):
            # sync ring during the attention stream; alternate sync/gpsimd
            # for the final drain (never scalar: it would steal exp time)
            eng = nc.gpsimd if (tail and out_dma_n[0] % 2) else nc.sync
            out_dma_n[0] += 1
            eng.dma_start(out=out_ext[i * 128:(i + 1) * 128, :], in_=out_full[i])

        def emit_exp(e, psc):
            # raw InstActivation with immediate bias/scale: skips the
            # per-partition bias-AP read the bass helper forces for Exp
            imm = lambda v: mybir.ImmediateValue(dtype=mybir.dt.float32, value=v)
            return nc.scalar.add_instruction(
                mybir.InstActivation(
                    name=nc.get_next_instruction_name(),
                    func=mybir.ActivationFunctionType.Exp,
                    ins=[nc.scalar.lower_ap(psc), imm(0.0), imm(SCALE), imm(0.0)],
                    outs=[nc.scalar.lower_ap(e)],
                )
            )

        def gen_tail(m, t4, cp):
            """Transpose/normalize pieces for segment (m, t4), reading the
            sbuf copy cp ([65,1024]: h0 | h1); one quantum per yield."""
            for c2 in range(4):
                for j in range(2):
                    pt = o_psum.tile([128, 65], BF16, tag="po", name=f"pt{m}_{t4}_{c2}_{j}")
                    nc.tensor.transpose(
                        pt,
                        cp[:, j * 512 + c2 * 128:j * 512 + (c2 + 1) * 128],
                        identity[0:65, 0:65],
                    )
                    ot = ot_sb.tile([128, 65], BF16, tag="ot", name=f"ot{m}_{t4}_{c2}_{j}")
                    nc.vector.tensor_copy(ot, pt)
                    rc = ot_sb.tile([128, 1], F32, tag="rc", name=f"rc{m}_{t4}_{c2}_{j}")
                    nc.vector.reciprocal(rc, ot[:, DH:DH + 1])
                    nc.vector.scalar_tensor_tensor(
                        out_full[t4 * 4 + c2][:, (2 * m + j) * DH:(2 * m + j + 1) * DH],
                        ot[:, 0:DH],
                        rc,
                        bv_bc[:, (2 * m + j) * DH:(2 * m + j + 1) * DH],
                        mybir.AluOpType.mult,
                        mybir.AluOpType.add,
                    )
                    if m == MB - 1 and j == 1:
                        emit_out_dma(t4 * 4 + c2, tail=final[0])
                    yield
            return

        # --- prelude: only q00/k00 gate the first chunk's scores ---
        for _ in gen_q_proj(0, 0):
            pass
        for _ in gen_k_proj(0, 0):
            pass

        # --- filler pacing: per-segment schedule of (due_chunk, fn), built
        # from explicit windows; tails drain via a dynamic spreader ---
        tails = []
        final = [False]
        tail_gen = [None, 0]  # generator, yields-left

        def drain_tail_step(k):
            for _ in range(k):
                if tail_gen[0] is None:
                    if not tails:
                        return
                    tail_gen[0] = tails.pop(0)
                    tail_gen[1] = 8
                try:
                    if tail_gen[1] > 0:
                        tail_gen[0].__next__()
                        tail_gen[1] -= 1
                    else:
                        deque(tail_gen[0], maxlen=0)
                        tail_gen[0] = None
                except StopIteration:
                    tail_gen[0] = None

        def make_sched(m, t4):
            sched = deque()

            def push(g, c0, c1, npops=4):
                total = npops + 1
                for i in range(total):
                    due = c0 + ((c1 - c0) * i) // (total - 1)
                    if i < npops:
                        sched.append((due, g.__next__))
                    else:
                        sched.append((due, lambda gg=g: deque(gg, maxlen=0)))

            # NOTE: every segment of pair m reads ALL of kT[m] (chunk c reads
            # sk cols c*128:(c+1)*128), so k(m, n) must complete inside the
            # FIRST segment of pair m by chunk 4n.  q(m, t4) is needed from
            # segment (m, t4) on.
            if t4 == 0:
                if m == 0:
                    push(gen_k_proj(0, 1), 0, 2)
                    push(gen_k_proj(0, 2), 3, 6)
                    push(gen_k_proj(0, 3), 7, 10)
                    push(gen_q_proj(0, 1), 11, 14)
                else:
                    push(gen_k_proj(m, 3), 1, 9)
                    push(gen_q_proj(m, 1), 10, 14)
            elif t4 == 1:
                if m == 0:
                    push(gen_q_proj(0, 2), 1, 8)
                else:
                    push(gen_q_proj(m, 2), 1, 7)
                    if m + 1 < MB:
                        push(gen_k_proj(m + 1, 1), 8, 15)
            elif t4 == 2:
                if m == 0:
                    push(gen_q_proj(0, 3), 1, 5)
                    push(gen_k_proj(1, 1), 6, 10)
                    push(gen_k_proj(1, 2), 11, 15)
                else:
                    push(gen_q_proj(m, 3), 1, 7)
                    if m + 1 < MB:
                        push(gen_k_proj(m + 1, 2), 8, 15)
            else:
                if m + 1 < MB:
                    push(gen_q_proj(m + 1, 0), 1, 6)
                    push(gen_k_proj(m + 1, 0), 7, 13)
            return sched

        pending = [None]

        for m in range(MB):
            for t4 in range(NT4):
                sched = make_sched(m, t4)
                po = o_psum.tile([128, 1024], F32, tag="po", name=f"po{m}_{t4}")
                sq = slice(t4 * 512, (t4 + 1) * 512)
                first_seg = (m == 0 and t4 == 0)

                def emit_av(c, e, po=po, m=m):
                    nc.tensor.matmul(
                        po[0:65, 0:512],
                        lhsT=vsb[c][:, 2 * m, :],
                        rhs=e[:, 0:512],
                        start=(c == 0), stop=(c == NS - 1),
                    )
                    nc.tensor.matmul(
                        po[0:65, 512:1024],
                        lhsT=vsb[c][:, 2 * m + 1, :],
                        rhs=e[:, 512:1024],
                        start=(c == 0), stop=(c == NS - 1),
                    )

                def finish_segment(po=po, m=m, t4=t4):
                    cp = cp_sb.tile([65, 1024], BF16, tag="cp", name=f"cp{m}_{t4}")
                    nc.vector.tensor_copy(cp, po[0:65, :])
                    tails.append(gen_tail(m, t4, cp))

                # two chunks per pipeline step: scores x2 (one K=64 island),
                # exp x2, then the previous step's AVs x4 (one full-array
                # island) -- halves the K64<->K128 PE mode-switch penalty
                for c0 in range(0, NS, 2):
                    es = []
                    for c in (c0, c0 + 1):
                        psc = s_psum.tile([128, 1024], F32, tag="psc", name=f"ps{m}_{t4}_{c}")
                        nc.tensor.matmul(
                            psc[:, 0:512],
                            lhsT=kT[m][0:64, c * 128:(c + 1) * 128],
                            rhs=qT[m][0:64, sq],
                            start=True, stop=True,
                        )
                        nc.tensor.matmul(
                            psc[:, 512:1024],
                            lhsT=kT[m][64:128, c * 128:(c + 1) * 128],
                            rhs=qT[m][64:128, sq],
                            start=True, stop=True,
                        )
                        e = e_pool.tile([128, 1024], BF16, tag="e", name=f"e{m}_{t4}_{c}")
                        emit_exp(e, psc)
                        es.append(e)
                    # tail transposes (K=65) right after the K=64 scores:
                    # they extend the small-K island instead of punching two
                    # extra mode switches into the full-array stream, and any
                    # PSUM-slot wait they incur is covered by the AVs below
                    if not first_seg and 1 <= c0 <= 13:
                        drain_tail_step(2 if len(tails) < 2 else 4)
                    # deferred AVs from the previous step (or the previous
                    # segment's last step + accumulator copy-out): their exps
                    # are done, so they never stall the PE queue
                    if pending[0] is not None:
                        pending[0]()
                        pending[0] = None
                    if first_seg:
                        # v(c0), v(c0+1) land ahead of their deferred AVs
                        for i in (c0, c0 + 1):
                            for _ in gen_v_proj(i):
                                pass
                    while sched and sched[0][0] <= c0 + 1:
                        sched.popleft()[1]()
                    if c0 < NS - 2:
                        def _p(es=es, c0=c0, emit_av=emit_av):
                            emit_av(c0, es[0])
                            emit_av(c0 + 1, es[1])
                        pending[0] = _p
                    else:
                        # retire the segment's last AVs inside the segment:
                        # the preceding v/filler PE work covers exp(14),(15)'s
                        # latency, and the next segment's first scores then
                        # enter a clean PE queue
                        emit_av(NS - 2, es[0])
                        emit_av(NS - 1, es[1])
                        finish_segment()

        if pending[0] is not None:
            pending[0]()
            pending[0] = None
        final[0] = True
        while tail_gen[0] is not None or tails:
            drain_tail_step(4)

    nc.compile()
    return nc


def _get_program():
    if "nc" not in _CACHE:
        _CACHE["nc"] = _build_program()
    return _CACHE["nc"]


def kernel(x, Wq, bq, Wk, bk, Wv, bv, _trace=False):
    bf = ml_dtypes.bfloat16
    x = np.asarray(x, dtype=np.float32)
    Wq = np.asarray(Wq, dtype=np.float32)
    Wk = np.asarray(Wk, dtype=np.float32)
    Wv = np.asarray(Wv, dtype=np.float32)
    bq = np.ascontiguousarray(np.asarray(bq, dtype=np.float32))
    bk = np.ascontiguousarray(np.asarray(bk, dtype=np.float32))
    bv = np.ascontiguousarray(np.asarray(bv, dtype=np.float32))

    nc = _get_program()

    def pack_w(W):
        # [D, DPC] -> [p=128, k=8, n] blob (8KB contiguous per partition)
        return np.ascontiguousarray(W.astype(bf).reshape(KD, 128, DPC).transpose(1, 0, 2))

    in_maps = []
    for c in range(N_CORES):
        b, g = c // 2, c % 2
        cols = slice(g * DPC, (g + 1) * DPC)
        # x[b].T is [D, S]; pack into 4 column-pieces of [p, j, 512]
        xt = x[b].T.astype(bf).reshape(KD, 128, NT4, 512)
        xt = np.ascontiguousarray(xt.transpose(2, 1, 0, 3))
        in_maps.append(
            {
                "xt": xt,
                "wq": pack_w(Wq[:, cols]),
                "wk": pack_w(Wk[:, cols]),
                "wv": pack_w(Wv[:, cols]),
                "bq": np.ascontiguousarray(bq[cols]),
                "bk": np.ascontiguousarray(bk[cols]),
                "bv": np.ascontiguousarray(bv[cols]),
            }
        )

    res = run_bass_kernel_spmd(nc, in_maps, core_ids=list(range(N_CORES)), trace=_trace)
    _CACHE["last_results"] = res

    out = np.empty((B, S, D), dtype=np.float32)
    for c in range(N_CORES):
        b, g = c // 2, c % 2
        out[b, :, g * DPC:(g + 1) * DPC] = res.results[c]["out"]
    return out


# revision 26
# speedup vs baseline: 1.1131x; 1.0055x over previous
"""Multi-head attention (B=4, S=2048, D=1024, H=16, Dh=64) on 8 TRN2 NeuronCores.

Sharding: core c -> batch b = c//2, head-group g = c%2 (8 heads, output cols
g*512:(g+1)*512).  Host ships x pre-transposed ([D, S]) and weights in bf16;
each core runs attention for its (batch, 8 heads) slice; host concatenates the
per-core [2048, 512] outputs.

v3 structure (vs v2):
- DMA: ~12 fat dma_starts on 3 engine rings (sync/scalar/gpsimd) instead of
  81 small ones on 2 rings -- descriptor-gen (618ns per start, serialized
  per ring) stops gating the load; x lands in 4 column pieces so the pair-0
  projections start ~6us in.
- PE warms up on a zero dummy tile (no x dependency) so HAM hits 2.4GHz
  before the first projection.
- Scores per chunk: 2 concurrent K=64 matmuls (PE row groups 0-63/64-127),
  N=512 each, into separate PSUM banks of one [128,1024] tile.
- AV is software-pipelined one chunk late: AV(c) is emitted after exp(c+1),
  so it never heads-of-line-blocks the PE queue waiting on exp(c); the
  previous segment's last AV + accumulator copy-out ride the same mechanism.
- Projection filler is paced by explicit per-generator deadline windows
  (chunk index within the segment), balanced so no segment except (0,0)
  exceeds the exp-rate PE budget; v projections run inline in segment (0,0).
- Tails (transpose/normalize of finished segments) drain via a dynamic
  spreader, never at segment boundaries.
"""

import numpy as np
import ml_dtypes
from collections import deque
from contextlib import ExitStack

import concourse.bass as bass
import concourse.bacc as bacc
import concourse.mybir as mybir
import concourse.tile as tile
from concourse.bass_utils import run_bass_kernel_spmd
from concourse.masks import make_identity

F32 = mybir.dt.float32
BF16 = mybir.dt.bfloat16

B, S, D = 4, 2048, 1024
H, DH = 16, 64
N_CORES = 8
HPC = 8          # heads per core
DPC = HPC * DH   # output cols per core = 512
SCALE = 1.0 / 32.0  # 1/sqrt(D)

KD = D // 128    # 8 contraction chunks over d_in
NS = S // 128    # 16 sk chunks
MB = HPC // 2    # 4 head pairs
NT4 = S // 512   # 4 sq tiles of 512

_CACHE = {}


def _build_program():
    nc = bacc.Bacc("TRN2", target_bir_lowering=False, debug=False)

    # x ships pre-packed as 4 column-pieces of [p=128, j=8 K-chunks, 512 s]
    # (contiguous per piece) and the weights as [p, k, n] blobs: one DMA
    # descriptor then covers 8KB per partition instead of 1KB per (p, chunk)
    # row, making the load bandwidth-bound instead of descriptor-bound.
    xt_ext = nc.dram_tensor("xt", [NT4, 128, KD, 512], BF16, kind="ExternalInput").ap()
    wq_ext = nc.dram_tensor("wq", [128, KD, DPC], BF16, kind="ExternalInput").ap()
    wk_ext = nc.dram_tensor("wk", [128, KD, DPC], BF16, kind="ExternalInput").ap()
    wv_ext = nc.dram_tensor("wv", [128, KD, DPC], BF16, kind="ExternalInput").ap()
    bq_ext = nc.dram_tensor("bq", [DPC], F32, kind="ExternalInput").ap()
    bk_ext = nc.dram_tensor("bk", [DPC], F32, kind="ExternalInput").ap()
    bv_ext = nc.dram_tensor("bv", [DPC], F32, kind="ExternalInput").ap()
    out_ext = nc.dram_tensor("out", [S, DPC], F32, kind="ExternalOutput").ap()

    with tile.TileContext(nc, pool_alloc_mode="queue") as tc, ExitStack() as ctx:
        singles = ctx.enter_context(tc.tile_pool(name="singles", bufs=1))

        # --- biases: one consolidated DMA each (tiny, land instantly) ---
        bqm = singles.tile([128, MB], F32, tag="bqm")
        nc.sync.dma_start(out=bqm, in_=bq_ext.rearrange("(m p) -> p m", p=128))
        bkm = singles.tile([128, MB], F32, tag="bkm")
        nc.sync.dma_start(out=bkm, in_=bk_ext.rearrange("(m p) -> p m", p=128))
        bv_f32 = singles.tile([1, DPC], F32, tag="bv_f32")
        nc.sync.dma_start(out=bv_f32, in_=bv_ext.rearrange("(o n) -> o n", o=1))
        bv_row = singles.tile([1, DPC], BF16, tag="bv_row")
        nc.vector.tensor_copy(bv_row, bv_f32)

        xT = singles.tile([128, KD, S], BF16, tag="xT")
        wq_sb = singles.tile([128, KD, DPC], BF16, tag="wq_sb")
        wk_sb = singles.tile([128, KD, DPC], BF16, tag="wk_sb")
        wv_sb = singles.tile([128, KD, DPC], BF16, tag="wv_sb")

        # identity/memsets first so no engine's iota queues behind a fat
        # DMA-descriptor-generation instruction
        identity = singles.tile([128, 128], BF16, tag="identity")
        make_identity(nc, identity)
        ones_row = singles.tile([1, 128], BF16, tag="ones_row")
        nc.vector.memset(ones_row, 1.0)
        dummy = singles.tile([128, 512], BF16, tag="dummy")
        nc.vector.memset(dummy, 0.0)

        # Pre-packed blobs make each descriptor cover 8KB per partition, so
        # the load is bandwidth-bound, not descriptor-bound.
        # arrival waves: [wq | x0] -> [wk | wv] -> [x2, x3 | x1]
        nc.sync.dma_start(out=wq_sb, in_=wq_ext)
        nc.sync.dma_start(out=wk_sb, in_=wk_ext)
        nc.sync.dma_start(out=xT[:, :, 1024:1536], in_=xt_ext[2])
        nc.sync.dma_start(out=xT[:, :, 1536:2048], in_=xt_ext[3])
        # ring: scalar (idle until the first exp anyway)
        nc.scalar.dma_start(out=xT[:, :, 0:512], in_=xt_ext[0])
        nc.scalar.dma_start(out=wv_sb, in_=wv_ext)
        nc.scalar.dma_start(out=xT[:, :, 512:1024], in_=xt_ext[1])

        # --- persistent sbuf tensors: pair-packed qT/kT (rows 0-63 even
        # head's projection cols, 64-127 odd head's), v natural + ones col ---
        qT = [singles.tile([128, S], BF16, tag=f"qT{m}", name=f"qT{m}") for m in range(MB)]
        kT = [singles.tile([128, S], BF16, tag=f"kT{m}", name=f"kT{m}") for m in range(MB)]
        vsb = [singles.tile([128, HPC, DH + 1], BF16, tag=f"v{i}", name=f"v{i}") for i in range(NS)]
        out_full = [singles.tile([128, DPC], F32, tag=f"of{i}", name=f"of{i}") for i in range(NS)]

        # --- psum pools: scores 2x[128,1024]f32 (4 banks) + shared
        # accumulator/projection pool (4 banks) ---
        o_psum = ctx.enter_context(tc.tile_pool(name="o_psum", bufs=2, space="PSUM"))
        s_psum = ctx.enter_context(tc.tile_pool(name="s_psum", bufs=2, space="PSUM"))

        e_pool = ctx.enter_context(tc.tile_pool(name="e_pool", bufs=6))
        cp_sb = ctx.enter_context(tc.tile_pool(name="cp_sb", bufs=4))
        ot_sb = ctx.enter_context(tc.tile_pool(name="ot_sb", bufs=8))

        # warm the PE clock (HAM) on the zero dummy (no DMA dependency): 8
        # cold pulses flip the SHORT window to 2.4GHz, 8 more bridge most of
        # the remaining DMA wait so q00 starts warm
        warm = o_psum.tile([128, 512], F32, tag="po", name="warm")
        for _ in range(16):
            nc.tensor.matmul(warm, lhsT=identity, rhs=dummy, start=True, stop=True)

        # bv replicated across partitions (one K=1 matmul): the v bias is
        # applied once in the tail -- attention weights sum to 1 exactly, so
        # out = sum(attn (v'+bv)) = sum(attn v') + bv
        bvp = o_psum.tile([128, DPC], F32, tag="po", name="bvp")
        nc.tensor.matmul(bvp, lhsT=ones_row, rhs=bv_row, start=True, stop=True)
        bv_bc = singles.tile([128, DPC], BF16, tag="bv_bc")
        nc.vector.tensor_copy(bv_bc, bvp)

        def gen_q_proj(m, n):
            """q projection for pair m, seq cols n*512:(n+1)*512."""
            sl = slice(n * 512, (n + 1) * 512)
            ps = o_psum.tile([128, 512], F32, tag="po", name=f"ppq{m}_{n}")
            for k in range(KD):
                nc.tensor.matmul(
                    ps,
                    lhsT=wq_sb[:, k, m * 128:(m + 1) * 128],
                    rhs=xT[:, k, sl],
                    start=(k == 0),
                    stop=(k == KD - 1),
                )
                if k % 2 == 1:
                    yield
            nc.vector.tensor_scalar_add(qT[m][:, sl], ps, bqm[:, m:m + 1])

        def gen_k_proj(m, n):
            sl = slice(n * 512, (n + 1) * 512)
            ps = o_psum.tile([128, 512], F32, tag="po", name=f"ppk{m}_{n}")
            for k in range(KD):
                nc.tensor.matmul(
                    ps,
                    lhsT=wk_sb[:, k, m * 128:(m + 1) * 128],
                    rhs=xT[:, k, sl],
                    start=(k == 0),
                    stop=(k == KD - 1),
                )
                if k % 2 == 1:
                    yield
            nc.vector.tensor_scalar_add(kT[m][:, sl], ps, bkm[:, m:m + 1])

        def gen_v_proj(i):
            ps = o_psum.tile([128, 512], F32, tag="po", name=f"vp{i}")
            for k in range(KD):
                nc.tensor.matmul(
                    ps,
                    lhsT=xT[:, k, i * 128:(i + 1) * 128],
                    rhs=wv_sb[:, k, :],
                    start=(k == 0),
                    stop=(k == KD - 1),
                )
                if k % 2 == 1:
                    yield
            nc.vector.tensor_copy(
                vsb[i][:, :, 0:DH], ps.rearrange("p (h d) -> p h d", h=HPC)
            )
            nc.vector.memset(vsb[i][:, :, DH:DH + 1], 1.0)

        out_dma_n = [0]

        def emit_out_dma(i, cols, tail=False):
            # sync ring during the attention stream; alternate sync/gpsimd
            # for the final drain (never scalar: it would steal exp time)
            eng = nc.gpsimd if (tail and out_dma_n[0] % 2) else nc.sync
            out_dma_n[0] += 1
            eng.dma_start(
                out=out_ext[i * 128:(i + 1) * 128, cols], in_=out_full[i][:, cols]
            )

        def emit_exp(e, psc):
            # raw InstActivation with immediate bias/scale: skips the
            # per-partition bias-AP read the bass helper forces for Exp
            imm = lambda v: mybir.ImmediateValue(dtype=mybir.dt.float32, value=v)
            return nc.scalar.add_instruction(
                mybir.InstActivation(
                    name=nc.get_next_instruction_name(),
                    func=mybir.ActivationFunctionType.Exp,
                    ins=[nc.scalar.lower_ap(psc), imm(0.0), imm(SCALE), imm(0.0)],
                    outs=[nc.scalar.lower_ap(e)],
                )
            )

        def gen_tail(m, t4, cp):
            """Transpose/normalize pieces for segment (m, t4), reading the
            sbuf copy cp ([65,1024]: h0 | h1); one quantum per yield."""
            for c2 in range(4):
                for j in range(2):
                    pt = o_psum.tile([128, 65], BF16, tag="po", name=f"pt{m}_{t4}_{c2}_{j}")
                    nc.tensor.transpose(
                        pt,
                        cp[:, j * 512 + c2 * 128:j * 512 + (c2 + 1) * 128],
                        identity[0:65, 0:65],
                    )
                    ot = ot_sb.tile([128, 65], BF16, tag="ot", name=f"ot{m}_{t4}_{c2}_{j}")
                    nc.vector.tensor_copy(ot, pt)
                    rc = ot_sb.tile([128, 1], F32, tag="rc", name=f"rc{m}_{t4}_{c2}_{j}")
                    nc.vector.reciprocal(rc, ot[:, DH:DH + 1])
                    nc.vector.scalar_tensor_tensor(
                        out_full[t4 * 4 + c2][:, (2 * m + j) * DH:(2 * m + j + 1) * DH],
                        ot[:, 0:DH],
                        rc,
                        bv_bc[:, (2 * m + j) * DH:(2 * m + j + 1) * DH],
                        mybir.AluOpType.mult,
                        mybir.AluOpType.add,
                    )
                    if m == MB - 2 and j == 1:
                        # pairs 0-2 complete: ship the bulk (cols 0:384) now
                        emit_out_dma(t4 * 4 + c2, slice(0, 3 * 128))
                    elif m == MB - 1 and j == 1:
                        emit_out_dma(t4 * 4 + c2, slice(3 * 128, DPC), tail=final[0])
                    yield
            return

        # --- prelude: only q00/k00 gate the first chunk's scores ---
        for _ in gen_q_proj(0, 0):
            pass
        for _ in gen_k_proj(0, 0):
            pass

        # --- filler pacing: per-segment schedule of (due_chunk, fn), built
        # from explicit windows; tails drain via a dynamic spreader ---
        tails = []
        final = [False]
        tail_gen = [None, 0]  # generator, yields-left

        def drain_tail_step(k):
            for _ in range(k):
                if tail_gen[0] is None:
                    if not tails:
                        return
                    tail_gen[0] = tails.pop(0)
                    tail_gen[1] = 8
                try:
                    if tail_gen[1] > 0:
                        tail_gen[0].__next__()
                        tail_gen[1] -= 1
                    else:
                        deque(tail_gen[0], maxlen=0)
                        tail_gen[0] = None
                except StopIteration:
                    tail_gen[0] = None

        def make_sched(m, t4):
            sched = deque()

            def push(g, c0, c1, npops=4):
                total = npops + 1
                for i in range(total):
                    due = c0 + ((c1 - c0) * i) // (total - 1)
                    if i < npops:
                        sched.append((due, g.__next__))
                    else:
                        sched.append((due, lambda gg=g: deque(gg, maxlen=0)))

            # NOTE: every segment of pair m reads ALL of kT[m] (chunk c reads
            # sk cols c*128:(c+1)*128), so k(m, n) must complete inside the
            # FIRST segment of pair m by chunk 4n.  q(m, t4) is needed from
            # segment (m, t4) on.
            if t4 == 0:
                if m == 0:
                    push(gen_k_proj(0, 1), 0, 2)
                    push(gen_k_proj(0, 2), 3, 6)
                    push(gen_k_proj(0, 3), 7, 10)
                    push(gen_q_proj(0, 1), 11, 14)
                else:
                    push(gen_k_proj(m, 3), 1, 9)
                    push(gen_q_proj(m, 1), 10, 14)
            elif t4 == 1:
                if m == 0:
                    push(gen_q_proj(0, 2), 1, 8)
                else:
                    push(gen_q_proj(m, 2), 1, 7)
                    if m + 1 < MB:
                        push(gen_k_proj(m + 1, 1), 8, 15)
            elif t4 == 2:
                if m == 0:
                    push(gen_q_proj(0, 3), 1, 5)
                    push(gen_k_proj(1, 1), 6, 10)
                    push(gen_k_proj(1, 2), 11, 15)
                else:
                    push(gen_q_proj(m, 3), 1, 7)
                    if m + 1 < MB:
                        push(gen_k_proj(m + 1, 2), 8, 15)
            else:
                if m + 1 < MB:
                    push(gen_q_proj(m + 1, 0), 1, 6)
                    push(gen_k_proj(m + 1, 0), 7, 13)
            return sched

        pending = [None]

        for m in range(MB):
            for t4 in range(NT4):
                sched = make_sched(m, t4)
                po = o_psum.tile([128, 1024], F32, tag="po", name=f"po{m}_{t4}")
                sq = slice(t4 * 512, (t4 + 1) * 512)
                first_seg = (m == 0 and t4 == 0)

                def emit_av(c, e, po=po, m=m):
                    nc.tensor.matmul(
                        po[0:65, 0:512],
                        lhsT=vsb[c][:, 2 * m, :],
                        rhs=e[:, 0:512],
                        start=(c == 0), stop=(c == NS - 1),
                    )
                    nc.tensor.matmul(
                        po[0:65, 512:1024],
                        lhsT=vsb[c][:, 2 * m + 1, :],
                        rhs=e[:, 512:1024],
                        start=(c == 0), stop=(c == NS - 1),
                    )

                def finish_segment(po=po, m=m, t4=t4):
                    cp = cp_sb.tile([65, 1024], BF16, tag="cp", name=f"cp{m}_{t4}")
                    nc.vector.tensor_copy(cp, po[0:65, :])
                    tails.append(gen_tail(m, t4, cp))

                # two chunks per pipeline step: scores x2 (one K=64 island),
                # exp x2, then the previous step's AVs x4 (one full-array
                # island) -- halves the K64<->K128 PE mode-switch penalty
                for c0 in range(0, NS, 2):
                    es = []
                    for c in (c0, c0 + 1):
                        psc = s_psum.tile([128, 1024], F32, tag="psc", name=f"ps{m}_{t4}_{c}")
                        nc.tensor.matmul(
                            psc[:, 0:512],
                            lhsT=kT[m][0:64, c * 128:(c + 1) * 128],
                            rhs=qT[m][0:64, sq],
                            start=True, stop=True,
                        )
                        nc.tensor.matmul(
                            psc[:, 512:1024],
                            lhsT=kT[m][64:128, c * 128:(c + 1) * 128],
                            rhs=qT[m][64:128, sq],
                            start=True, stop=True,
                        )
                        e = e_pool.tile([128, 1024], BF16, tag="e", name=f"e{m}_{t4}_{c}")
                        emit_exp(e, psc)
                        es.append(e)
                    # tail transposes (K=65) right after the K=64 scores:
                    # they extend the small-K island instead of punching two
                    # extra mode switches into the full-array stream, and any
                    # PSUM-slot wait they incur is covered by the AVs below
                    if not first_seg and 1 <= c0 <= 13:
                        drain_tail_step(2 if len(tails) < 2 else 4)
                    # deferred AVs from the previous step (or the previous
                    # segment's last step + accumulator copy-out): their exps
                    # are done, so they never stall the PE queue
                    if pending[0] is not None:
                        pending[0]()
                        pending[0] = None
                    if first_seg:
                        # v(c0), v(c0+1) land ahead of their deferred AVs
                        for i in (c0, c0 + 1):
                            for _ in gen_v_proj(i):
                                pass
                    while sched and sched[0][0] <= c0 + 1:
                        sched.popleft()[1]()
                    if c0 < NS - 2:
                        def _p(es=es, c0=c0, emit_av=emit_av):
                            emit_av(c0, es[0])
                            emit_av(c0 + 1, es[1])
                        pending[0] = _p
                    else:
                        # retire the segment's last AVs inside the segment:
                        # the preceding v/filler PE work covers exp(14),(15)'s
                        # latency, and the next segment's first scores then
                        # enter a clean PE queue
                        emit_av(NS - 2, es[0])
                        emit_av(NS - 1, es[1])
                        finish_segment()

        if pending[0] is not None:
            pending[0]()
            pending[0] = None
        final[0] = True
        while tail_gen[0] is not None or tails:
            drain_tail_step(4)

    nc.compile()
    return nc


def _get_program():
    if "nc" not in _CACHE:
        _CACHE["nc"] = _build_program()
    return _CACHE["nc"]


def kernel(x, Wq, bq, Wk, bk, Wv, bv, _trace=False):
    bf = ml_dtypes.bfloat16
    x = np.asarray(x, dtype=np.float32)
    Wq = np.asarray(Wq, dtype=np.float32)
    Wk = np.asarray(Wk, dtype=np.float32)
    Wv = np.asarray(Wv, dtype=np.float32)
    bq = np.ascontiguousarray(np.asarray(bq, dtype=np.float32))
    bk = np.ascontiguousarray(np.asarray(bk, dtype=np.float32))
    bv = np.ascontiguousarray(np.asarray(bv, dtype=np.float32))

    nc = _get_program()

    def pack_w(W):
        # [D, DPC] -> [p=128, k=8, n] blob (8KB contiguous per partition)
        return np.ascontiguousarray(W.astype(bf).reshape(KD, 128, DPC).transpose(1, 0, 2))

    in_maps = []
    for c in range(N_CORES):
        b, g = c // 2, c % 2
        cols = slice(g * DPC, (g + 1) * DPC)
        # x[b].T is [D, S]; pack into 4 column-pieces of [p, j, 512]
        xt = x[b].T.astype(bf).reshape(KD, 128, NT4, 512)
        xt = np.ascontiguousarray(xt.transpose(2, 1, 0, 3))
        in_maps.append(
            {
                "xt": xt,
                "wq": pack_w(Wq[:, cols]),
                "wk": pack_w(Wk[:, cols]),
                "wv": pack_w(Wv[:, cols]),
                "bq": np.ascontiguousarray(bq[cols]),
                "bk": np.ascontiguousarray(bk[cols]),
                "bv": np.ascontiguousarray(bv[cols]),
            }
        )

    res = run_bass_kernel_spmd(nc, in_maps, core_ids=list(range(N_CORES)), trace=_trace)
    _CACHE["last_results"] = res

    out = np.empty((B, S, D), dtype=np.float32)
    for c in range(N_CORES):
        b, g = c // 2, c % 2
        out[b, :, g * DPC:(g + 1) * DPC] = res.results[c]["out"]
    return out
